# revision 14
# baseline (speedup 1.0000x reference)
"""Multi-Head Latent Attention (DeepSeek-style MLA) on 8 TRN2 NeuronCores.

Sharding: core c handles batch b = c//2 and query rows [ (c%2)*S/2, (c%2+1)*S/2 ).
Each core computes the full KV-side projections for its batch (duplicated between
the two cores sharing a batch) and the Q-side projections / attention / output
projection for its query half. No collectives; the host gathers the 8 output
shards.

Layout strategy: activations are kept feature-major ("transposed", [feature, seq])
so every matmul's contraction dim lands on SBUF partitions. Attention output is
produced directly as attT[h*128+d, q] (v as stationary operand, expT as moving),
which is exactly the lhsT layout the output projection needs - no PE transposes
anywhere. Softmax skips the max-subtraction (scores here are O(1); exp is safe)
and the denominator comes from an all-ones stationary matmul over expT.

RoPE is folded into companion weight matrices host-side:
  rope(x)[2i]   = x[2i] cos_i - x[2i+1] sin_i
  rope(x)[2i+1] = x[2i+1] cos_i + x[2i] sin_i
so with xr = x @ Wr where Wr[:,2i] = -W[:,2i+1], Wr[:,2i+1] = W[:,2i]:
  rope(x @ W) = (x @ W) * cosP + (x @ Wr) * sinP   (pure elementwise).

fp8 DoubleRow GEMMs: every big matmul except att@v runs in fp8e4 with
MatmulPerfMode.DoubleRow (two 128-row contraction tiles per instruction at
0.5 cycles/output-row = 4x bf16 FLOP rate). Accuracy is recovered with
hi/lo compensation: each operand X is split into fp8 X_hi = fp8(X) and
X_lo = fp8(X - X_hi); the product uses three DoubleRow matmuls
(hi*hi + lo*hi + hi*lo, dropping the ~2^-8 lo*lo term), i.e. 1.5 passes
where bf16 needs 2 -> 25% fewer PE cycles at bf16-level accuracy
(measured slightly better than bf16 on HW). The softmax denominator runs
in RAW fp8 (error averages out over 2048 keys): exp pair-sums are cast to
fp8 and fed 4-at-a-time to a DoubleRow ones-matmul -> 4x fewer Z cycles.

fp8e4's narrow range (subnormals below 2^-6) forces power-of-2 scaling on
every operand into the [~0.25, 30] band: x*4, down-proj weights*256,
c_kv/c_q stored *4, k/kr stored *8, q/q_rot stored *64 (SCALER folded in),
up-proj weights *2048, attn-out *128, W_o *256. Scales cancel at the exp
(activation scale 2^-9), the ones-matmul (ones value 8 folds v's 1024 into
attn-out's 128), and the final output eviction (*2^-15).

Scheduling notes (inherited from the bf16 baseline, each measured on HW):
- A DMA occupies its issuing engine's queue for the whole transfer, so
  traffic is spread: P1 weights + tables on the ACT ring, activation
  streams / attd / outputs / k_rot copies on the SP ring, attention-phase
  weight prefetches and SBUF shuffles on the gpsimd SWDGE ring.
- kT/v for head h+1 are produced during head h so their PSUM evictions
  sit ahead of the slow [128,512] reciprocals in engine FIFOs.
- psS has 3 banks so the scheduler can hoist the next kc's score matmul
  over the ~800ns exp latency; phase-1/3 chains share its tag, the
  produce_v/k chains take the single psA bank, att@v and the softmax
  denominator accumulate in psG/psZ (2+2).
"""

import sys
import numpy as np

sys.path.insert(0, "/opt/trn_rl_repo")

from contextlib import ExitStack  # noqa: E402

import concourse.bass as bass  # noqa: E402
import concourse.mybir as mybir  # noqa: E402
import concourse.tile as tile  # noqa: E402

F32 = mybir.dt.float32
BF = mybir.dt.bfloat16
FP8 = mybir.dt.float8e4
AF = mybir.ActivationFunctionType
ALU = mybir.AluOpType
DR = mybir.MatmulPerfMode.DoubleRow

# Max sync-waits walrus CoreV3 codegen accepts on one instruction. The stock
# TileContext tail-drain attaches one wait per outstanding semaphore to a
# single Drain, which this walrus build rejects ("Too many sync wait
# commands"); split across several drains instead.
_MAX_WAITS_PER_INST = 1


def _split_excess_waits_json(bir_json):
    """Walrus CoreV3 codegen rejects instructions carrying more than one
    sync-wait. Tile freely attaches several. Rewrite the BIR: keep one wait on
    the instruction, move the rest onto NoOps inserted just before it on the
    same engine (a same-engine wait that fires earlier is strictly safe).
    Updates are left untouched - they must fire at instruction completion."""
    import orjson

    bir = orjson.loads(bir_json)
    n = 0
    for fn in bir.get("functions", []):
        for bb in fn.get("blocks", []):
            out = []
            for inst in bb.get("instructions", []):
                si = inst.get("sync_info")
                waits = (si or {}).get("on_wait") or []
                if len(waits) > _MAX_WAITS_PER_INST:
                    keep = waits[-_MAX_WAITS_PER_INST:]
                    for w in waits[:-_MAX_WAITS_PER_INST]:
                        out.append({
                            "name": f"I-WS{n}",
                            "opcode": "NoOp",
                            "engine": inst["engine"],
                            "ins": [],
                            "outs": [],
                            "sync_info": {"on_update": [], "on_wait": [w]},
                        })
                        n += 1
                    si["on_wait"] = keep
                out.append(inst)
            bb["instructions"] = out
    return orjson.dumps(bir)


_COMPILE_HOOKED = False


def _install_wait_split_hook():
    """Wrap compile_bir_kernel (both the bass_utils global and the name
    bass2jax imported) so every BIR headed to walrus gets the wait split."""
    global _COMPILE_HOOKED
    if _COMPILE_HOOKED:
        return
    from concourse import bass2jax, bass_utils

    orig = bass_utils.compile_bir_kernel

    def hooked(bir_json, tmpdir, neff_name="file.neff"):
        return orig(_split_excess_waits_json(bir_json), tmpdir, neff_name=neff_name)

    bass_utils.compile_bir_kernel = hooked
    bass2jax.compile_bir_kernel = hooked
    _COMPILE_HOOKED = True


class SplitDrainTileContext(tile.TileContext):
    def _drain_and_barrier(self, tick_clock, wait_clock):
        from concourse.tile_scheduler import N_PROCS
        from concourse.vector_clock import ScopedClock, VectorClock

        g = tick_clock.global_clock
        vals = [g[p] for p in range(N_PROCS)]
        nz = [p for p in range(N_PROCS) if vals[p] > 0]
        groups = [nz[i:i + _MAX_WAITS_PER_INST]
                  for i in range(0, len(nz), _MAX_WAITS_PER_INST)] or [[]]
        for grp in groups:
            sub = VectorClock([vals[p] if p in grp else 0 for p in range(N_PROCS)])
            drain_inst = self.nc.sync.drain()
            wait_clock.add_sem_waits(drain_inst.ins, ScopedClock({None: sub}))

        self.nc.all_engine_barrier()
        assert self.sems is not None
        popped = self.nc._tile_sem_poison_stack.pop()
        assert popped is self._sem_poison
        self.nc.clear_and_free_semaphores(list(self.sems.allocated().values()))
        self.nc.all_engine_barrier()


# ----------------------------------------------------------------------------
# Config
# ----------------------------------------------------------------------------

class Cfg:
    def __init__(self, E=2048, DM=2048, H=16, DC=512, DC1=1536, S=2048, Q=1024,
                 QT=512, bf16=True):
        self.E, self.DM, self.H, self.DC, self.DC1 = E, DM, H, DC, DC1
        self.S, self.Q, self.QT = S, Q, QT
        self.DR = 64          # rotary dim (fixed by the problem)
        self.DH = 128         # nope head dim (fixed: DM // H)
        self.bf16 = bf16      # kept for test.py compat; kernel is fp8
        assert DM == H * self.DH and H % 2 == 0
        assert E % 256 == 0 and DC % 256 == 0 and DC1 % 256 == 0
        assert S % 128 == 0
        assert Q % QT == 0 and Q % 128 == 0 and QT <= 512
        self.EC = E // 128        # embed chunks
        self.EP = E // 256        # embed chunk pairs (DoubleRow)
        self.CC = DC // 128       # c_kv chunks
        self.CP = DC // 256
        self.C1C = DC1 // 128     # c_q chunks
        self.C1P = DC1 // 256
        self.KC = S // 128        # key chunks (128-wide)
        self.ST = min(512, S)     # seq tile for phase 1
        self.STN = S // self.ST
        self.NT = min(512, S)     # kT free tile
        self.NTN = S // self.NT
        self.QTN = Q // QT
        self.MT = min(512, DM)    # out-proj free tile
        self.MTN = DM // self.MT
        self.QON = Q // 128       # out-proj q tiles
        self.HP = H // 2          # head pairs (DoubleRow out-proj contraction)


FULL = Cfg()

# power-of-2 operand scales (see module docstring)
SX = 4.0            # x
SW1 = 256.0         # wdkv, wdq, wrk
SCKV = 4.0          # stored c_kv / c_q
E1 = SCKV / (SX * SW1)       # 2^-8 down-proj eviction
SWU = 256.0         # wuk, wuv
SK = 8.0            # stored k / roped k_rot
EK = SK / (SCKV * SWU)       # 2^-7 k eviction
EKR = SK / (SX * SW1)        # 2^-7 k_rot eviction
SV = SCKV * SWU     # v stored scale (bf16) = 1024
SWQ = 2048.0        # wuq, wrq, wrqr (SCALER folded in)
SQ = 64.0           # stored q / q_rot
EQ = SQ / (SCKV * SWQ)       # 2^-7 q eviction
ESC = 1.0 / (SQ * SK)        # 2^-9 exp pre-scale
SA = 128.0          # stored attn-out
ONEV = SV / SA      # 8.0 - ones-matmul value folds v scale into attn scale
SWO = 256.0         # wo
EO = 1.0 / (SA * SWO)        # 2^-15 final eviction


# ----------------------------------------------------------------------------
# Program builder (single-core SPMD program)
# ----------------------------------------------------------------------------

def build_program(cfg: Cfg, has_buv=True, has_bo=True, nz=None):
    c = cfg
    nz = nz or {}
    nzkv = nz.get("bdkv", False)
    nzdq = nz.get("bdq", False)
    nzuq = nz.get("buq", False)
    nzrq = nz.get("brq", False)
    nzrk = nz.get("brk", False)
    nzuk = nz.get("buk", False)
    nc = bass.Bass()

    # -- DRAM parameters -----------------------------------------------------
    xth = nc.dram_tensor("xth", [c.E, c.S], FP8, kind="ExternalInput")
    xtl = nc.dram_tensor("xtl", [c.E, c.S], FP8, kind="ExternalInput")
    xqh = nc.dram_tensor("xqh", [c.E, c.Q], FP8, kind="ExternalInput")
    xql = nc.dram_tensor("xql", [c.E, c.Q], FP8, kind="ExternalInput")
    cosq = nc.dram_tensor("cosq", [128, c.Q], F32, kind="ExternalInput")
    sinq = nc.dram_tensor("sinq", [128, c.Q], F32, kind="ExternalInput")
    # rows 0:64 cos table, rows 64:128 sin table (packed for the fused k-rope)
    cossink = nc.dram_tensor("cossink", [128, c.S], F32, kind="ExternalInput")
    wdqh = nc.dram_tensor("wdqh", [c.E, c.DC1], FP8, kind="ExternalInput")
    wdql = nc.dram_tensor("wdql", [c.E, c.DC1], FP8, kind="ExternalInput")
    bdq = nc.dram_tensor("bdq", [c.DC1], F32, kind="ExternalInput")
    wdkvh = nc.dram_tensor("wdkvh", [c.E, c.DC], FP8, kind="ExternalInput")
    wdkvl = nc.dram_tensor("wdkvl", [c.E, c.DC], FP8, kind="ExternalInput")
    bdkv = nc.dram_tensor("bdkv", [c.DC], F32, kind="ExternalInput")
    wuqh = nc.dram_tensor("wuqh", [c.DC1, c.DM], FP8, kind="ExternalInput")
    wuql = nc.dram_tensor("wuql", [c.DC1, c.DM], FP8, kind="ExternalInput")
    buq = nc.dram_tensor("buq", [c.DM], F32, kind="ExternalInput")
    wrqh = nc.dram_tensor("wrqh", [c.DC1, c.H * c.DR], FP8, kind="ExternalInput")
    wrql = nc.dram_tensor("wrql", [c.DC1, c.H * c.DR], FP8, kind="ExternalInput")
    brq = nc.dram_tensor("brq", [c.H * c.DR], F32, kind="ExternalInput")
    wrqrh = nc.dram_tensor("wrqrh", [c.DC1, c.H * c.DR], FP8, kind="ExternalInput")
    wrqrl = nc.dram_tensor("wrqrl", [c.DC1, c.H * c.DR], FP8, kind="ExternalInput")
    brqr = nc.dram_tensor("brqr", [c.H * c.DR], F32, kind="ExternalInput")
    wrkh = nc.dram_tensor("wrkh", [c.E, 2 * c.DR], FP8, kind="ExternalInput")
    wrkl = nc.dram_tensor("wrkl", [c.E, 2 * c.DR], FP8, kind="ExternalInput")
    brk = nc.dram_tensor("brk", [2 * c.DR], F32, kind="ExternalInput")
    wukh = nc.dram_tensor("wukh", [c.DC, c.DM], FP8, kind="ExternalInput")
    wukl = nc.dram_tensor("wukl", [c.DC, c.DM], FP8, kind="ExternalInput")
    buk = nc.dram_tensor("buk", [c.DM], F32, kind="ExternalInput")
    wuvh = nc.dram_tensor("wuvh", [c.DC, c.DM], FP8, kind="ExternalInput")
    wuvl = nc.dram_tensor("wuvl", [c.DC, c.DM], FP8, kind="ExternalInput")
    buv = nc.dram_tensor("buv", [c.DM], BF, kind="ExternalInput")
    woh = nc.dram_tensor("woh", [c.DM, c.DM], FP8, kind="ExternalInput")
    wol = nc.dram_tensor("wol", [c.DM, c.DM], FP8, kind="ExternalInput")
    bo = nc.dram_tensor("bo", [c.DM], BF, kind="ExternalInput")
    ones_d = nc.dram_tensor("ones_in", [256, 128], FP8, kind="ExternalInput")
    ones_bf_d = nc.dram_tensor("ones_bf", [1, 128], BF, kind="ExternalInput")
    out = nc.dram_tensor("out", [c.Q, c.DM], F32, kind="ExternalOutput")
    attdh = nc.dram_tensor("attTh_scratch", [c.DM, c.Q], FP8)
    attdl = nc.dram_tensor("attTl_scratch", [c.DM, c.Q], FP8)

    prr = lambda t: t.rearrange("(pair two p) m -> p pair two m", p=128, two=2)  # noqa: E731

    with SplitDrainTileContext(nc) as tc, ExitStack() as ctx:
        # weights / tables / small SBUF-SBUF shuffles ride the ACT HWDGE
        # ring; activation streams, attd, outputs and the k_rot pair copies
        # ride the SP ring; the attention-phase weight prefetches and qr
        # shuffles ride the gpsimd SWDGE ring.
        wdma = nc.scalar.dma_start
        adma = nc.sync.dma_start
        gdma = nc.gpsimd.dma_start

        # -- persistent pools ------------------------------------------------
        consts = ctx.enter_context(tc.tile_pool(name="consts", bufs=1))
        res = ctx.enter_context(tc.tile_pool(name="res", bufs=1))

        # c_kv^T hi/lo, pair-packed for DoubleRow: [p, cc-pair, ktile, s]
        ckvh = res.tile([128, c.CP, 2, c.S], FP8, tag="ckvh")
        ckvl = res.tile([128, c.CP, 2, c.S], FP8, tag="ckvl")
        # roped k_rot^T hi/lo (rows 64:128 dup'd finite, they hit zero q rows)
        krph = res.tile([128, c.S], FP8, tag="krph")
        krpl = res.tile([128, c.S], FP8, tag="krpl")
        # q side, DoubleRow-ready: ktile 0 = q (scaled), ktile 1 = roped q_rot
        # in rows 0:64 with rows 64:128 zeroed
        qnrh = res.tile([128, 2, c.H, c.Q], FP8, tag="qnrh")
        qnrl = res.tile([128, 2, c.H, c.Q], FP8, tag="qnrl")

        def load_pcol(name, vec, n):
            # [n*128] dram vector -> [128, n] sbuf (per-partition scalars)
            t = consts.tile([128, n], F32, tag=name)
            wdma(out=t, in_=vec.rearrange("(c p) -> p c", p=128))
            return t

        # PSUM pools (8 banks total: 1+3+2+2)
        psA = ctx.enter_context(tc.tile_pool(name="psA", bufs=1, space="PSUM"))
        psS = ctx.enter_context(tc.tile_pool(name="psS", bufs=3, space="PSUM"))
        psG = ctx.enter_context(tc.tile_pool(name="psG", bufs=2, space="PSUM"))
        psZ = ctx.enter_context(tc.tile_pool(name="psZ", bufs=2, space="PSUM"))

        # head 0/1 attention weights live below the phase pools so their
        # DMAs (issued during 1c) never wait on an aliased zone
        hw = ctx.enter_context(tc.tile_pool(name="hw", bufs=3))

        # scratch pool for nonzero-bias 3-op evictions (skipped when all
        # biases are zero - the common case - to save SBUF)
        any_nz = nzkv or nzdq or nzuq or nzrq or nzuk
        bt_pool = (ctx.enter_context(tc.tile_pool(name="btmp", bufs=2))
                   if any_nz else None)

        def ev_hilo(hi_ap, lo_ap, ps, scale, bias_col, n, hi_act=False):
            """hi = fp8(ps*scale [+ bias]); lo = fp8((ps*scale [+ bias]) - hi).
            Zero-bias: 2 ops (hi on DVE or ACT, lo on DVE - gpsimd cannot
            read PSUM). Nonzero bias: 3 ops through an f32 staging tile."""
            if bias_col is None:
                if hi_act:
                    nc.scalar.mul(hi_ap, ps, scale)
                else:
                    nc.vector.tensor_scalar_mul(hi_ap, ps, scale)
                nc.vector.scalar_tensor_tensor(lo_ap, ps, scale, hi_ap,
                                               ALU.mult, ALU.subtract)
            else:
                tmpb = bt_pool.tile([128, n], F32, tag="btmp")
                nc.vector.tensor_scalar(tmpb, ps, scale, bias_col,
                                        ALU.mult, ALU.add)
                if hi_act:
                    nc.scalar.copy(hi_ap, tmpb)
                else:
                    nc.vector.tensor_copy(hi_ap, tmpb)
                nc.gpsimd.tensor_sub(lo_ap, tmpb, hi_ap)

        # ==================================================================
        # Phase 1a: c_kv^T and roped k_rot^T over the full sequence
        # ==================================================================
        with tc.tile_pool(name="p1ax", bufs=2 * c.EC + 2) as p1ax, \
             tc.tile_pool(name="p1aw", bufs=c.EP) as p1aw, \
             tc.tile_pool(name="p1am", bufs=1) as p1am, \
             tc.tile_pool(name="p1at", bufs=4) as p1at:

            # ACT-ring issue order = need order: the wdkv pair tiles gate the
            # first matmul chain, wrk/cossink the k_rot tail; everything else
            # is needed phases later.
            wdkvh_t, wdkvl_t, wrkh_t, wrkl_t = [], [], [], []
            for p in range(c.EP):
                wdkvh_t.append(p1aw.tile([128, 2, c.DC], FP8, tag="wdkvh",
                                         name=f"wdkvh{p}"))
                wdkvl_t.append(p1aw.tile([128, 2, c.DC], FP8, tag="wdkvl",
                                         name=f"wdkvl{p}"))
            for p in range(c.EP):
                wdma(out=wdkvh_t[p], in_=prr(wdkvh)[:, p])
                wdma(out=wdkvl_t[p], in_=prr(wdkvl)[:, p])
            bdkv_sb = load_pcol("bdkv", bdkv, c.CC) if nzkv else None
            for p in range(c.EP):
                wrkh_t.append(p1aw.tile([128, 2, 2 * c.DR], FP8, tag="wrkh",
                                        name=f"wrkh{p}"))
                wrkl_t.append(p1aw.tile([128, 2, 2 * c.DR], FP8, tag="wrkl",
                                        name=f"wrkl{p}"))
            for p in range(c.EP):
                wdma(out=wrkh_t[p], in_=prr(wrkh)[:, p])
                wdma(out=wrkl_t[p], in_=prr(wrkl)[:, p])
            brk_sb = load_pcol("brk", brk, 1) if nzrk else None
            coss_sb = p1am.tile([128, c.S], F32, tag="coss")
            wdma(out=coss_sb, in_=cossink[:, :])
            bdq_sb = load_pcol("bdq", bdq, c.C1C) if nzdq else None
            buq_sb = load_pcol("buq", buq, c.H) if nzuq else None
            brq_sb = load_pcol("brq", brq, c.H // 2) if nzrq else None
            brqr_sb = load_pcol("brqr", brqr, c.H // 2) if nzrq else None
            buk_sb = load_pcol("buk", buk, c.H) if nzuk else None
            buv_sb = bo_sb = ones1 = None
            if has_buv:
                buv_sb = consts.tile([1, c.DM], BF, tag="buv")
                wdma(out=buv_sb, in_=buv[:].unsqueeze(0))
            if has_bo:
                bo_sb = consts.tile([1, c.DM], BF, tag="bo")
                wdma(out=bo_sb, in_=bo[:].unsqueeze(0))
            ones2 = consts.tile([128, 2, 128], FP8, tag="ones2")
            wdma(out=ones2, in_=ones_d.rearrange("(two p) m -> p two m", p=128))
            if has_buv or has_bo:
                ones1 = consts.tile([1, 128], BF, tag="ones1")
                wdma(out=ones1, in_=ones_bf_d[:, :])

            for st in range(c.STN):
                ssl = bass.ts(st, c.ST)
                xh, xl = [], []
                for p in range(c.EP):
                    th = p1ax.tile([128, 2, c.ST], FP8, tag="x")
                    adma(out=th, in_=prr(xth)[:, p, :, ssl])
                    tl = p1ax.tile([128, 2, c.ST], FP8, tag="x")
                    adma(out=tl, in_=prr(xtl)[:, p, :, ssl])
                    xh.append(th)
                    xl.append(tl)
                for ct in range(c.CC):
                    csl = bass.ts(ct, 128)
                    ps = psS.tile([128, c.ST], F32, tag="s")
                    for p in range(c.EP):
                        nc.tensor.matmul(ps, wdkvh_t[p][:, :, csl], xh[p],
                                         start=(p == 0), stop=False, perf_mode=DR)
                    for p in range(c.EP):
                        nc.tensor.matmul(ps, wdkvl_t[p][:, :, csl], xh[p],
                                         start=False, stop=False, perf_mode=DR)
                    for p in range(c.EP):
                        nc.tensor.matmul(ps, wdkvh_t[p][:, :, csl], xl[p],
                                         start=False, stop=(p == c.EP - 1),
                                         perf_mode=DR)
                    ev_hilo(ckvh[:, ct // 2, ct % 2, ssl],
                            ckvl[:, ct // 2, ct % 2, ssl], ps, E1,
                            bdkv_sb[:, ct:ct + 1] if nzkv else None, c.ST)
                # k_rot: one DoubleRow stationary covers A rows (0:64, cos
                # part) and companion Ar rows (64:128, sin part) in one psum
                ps = psS.tile([128, c.ST], F32, tag="s")
                for p in range(c.EP):
                    nc.tensor.matmul(ps, wrkh_t[p], xh[p],
                                     start=(p == 0), stop=False, perf_mode=DR)
                for p in range(c.EP):
                    nc.tensor.matmul(ps, wrkl_t[p], xh[p],
                                     start=False, stop=False, perf_mode=DR)
                for p in range(c.EP):
                    nc.tensor.matmul(ps, wrkh_t[p], xl[p],
                                     start=False, stop=(p == c.EP - 1),
                                     perf_mode=DR)
                tmp = p1at.tile([128, c.ST], F32, tag="ktmp")
                if nzrk:
                    nc.vector.scalar_tensor_tensor(
                        tmp[0:64, :], ps[0:64, :], EKR, brk_sb[0:64, :],
                        ALU.mult, ALU.add)
                    nc.vector.tensor_mul(tmp[0:64, :], tmp[0:64, :],
                                         coss_sb[0:64, ssl])
                    nc.vector.scalar_tensor_tensor(
                        tmp[64:128, :], ps[64:128, :], EKR, brk_sb[64:128, :],
                        ALU.mult, ALU.add)
                    nc.vector.tensor_mul(tmp[64:128, :], tmp[64:128, :],
                                         coss_sb[64:128, ssl])
                else:
                    nc.vector.scalar_tensor_tensor(
                        tmp[0:64, :], ps[0:64, :], EKR, coss_sb[0:64, ssl],
                        ALU.mult, ALU.mult)
                    nc.vector.scalar_tensor_tensor(
                        tmp[64:128, :], ps[64:128, :], EKR,
                        coss_sb[64:128, ssl], ALU.mult, ALU.mult)
                tmp2 = p1at.tile([64, c.ST], F32, tag="ktmp2")
                gdma(out=tmp2, in_=tmp[64:128, :])
                nc.vector.tensor_add(tmp[0:64, :], tmp[0:64, :], tmp2)
                nc.gpsimd.tensor_copy(krph[0:64, ssl], tmp[0:64, :])
                nc.vector.scalar_tensor_tensor(krpl[0:64, ssl], tmp[0:64, :],
                                               1.0, krph[0:64, ssl],
                                               ALU.mult, ALU.subtract)
            # duplicate kr rows: rows 64:128 are the stationary rows that
            # multiply the q side's zero rows - any finite value works, a
            # copy is the cheapest way to guarantee finite.
            gdma(out=krph[64:128, :], in_=krph[0:64, :])
            gdma(out=krpl[64:128, :], in_=krpl[0:64, :])

        # zero the q_rot pad rows off the k_rot tail's critical path; the
        # score matmuls (attention) are the only readers
        nc.gpsimd.memset(qnrh[64:128, 1, :, :], 0.0)
        nc.gpsimd.memset(qnrl[64:128, 1, :, :], 0.0)

        # pre-issue head 0/1 attention weights: the hw zone aliases nothing,
        # so these flow on the ACT ring during 1b/1c
        wuvh_p = hw.tile([128, c.CP, 2, 256], FP8, tag="wuvh", name="wuvh_pre")
        wdma(out=wuvh_p, in_=prr(wuvh)[:, :, :, 0:256])
        wuvl_p = hw.tile([128, c.CP, 2, 256], FP8, tag="wuvl", name="wuvl_pre")
        wdma(out=wuvl_p, in_=prr(wuvl)[:, :, :, 0:256])
        wukh_p0 = hw.tile([128, c.CP, 2, 128], FP8, tag="wukh", name="wukh_pre0")
        wdma(out=wukh_p0, in_=prr(wukh)[:, :, :, 0:128])
        wukl_p0 = hw.tile([128, c.CP, 2, 128], FP8, tag="wukl", name="wukl_pre0")
        wdma(out=wukl_p0, in_=prr(wukl)[:, :, :, 0:128])
        wukh_p1 = hw.tile([128, c.CP, 2, 128], FP8, tag="wukh", name="wukh_pre1")
        wdma(out=wukh_p1, in_=prr(wukh)[:, :, :, 128:256])
        wukl_p1 = hw.tile([128, c.CP, 2, 128], FP8, tag="wukl", name="wukl_pre1")
        wdma(out=wukl_p1, in_=prr(wukl)[:, :, :, 128:256])

        with tc.tile_pool(name="p1bx", bufs=2 * c.QTN * c.EP + 2) as p1bx, \
             tc.tile_pool(name="p1bw", bufs=2) as p1bw:
            # 1b's activations: fresh zone, so these queue dep-free on the
            # SP ring right behind 1a's x stream
            xqs = {}
            for qt in range(c.QTN):
                qsl = bass.ts(qt, c.QT)
                for p in range(c.EP):
                    th = p1bx.tile([128, 2, c.QT], FP8, tag="xq")
                    adma(out=th, in_=prr(xqh)[:, p, :, qsl])
                    tl = p1bx.tile([128, 2, c.QT], FP8, tag="xq")
                    adma(out=tl, in_=prr(xql)[:, p, :, qsl])
                    xqs[qt, p] = (th, tl)

            # ==============================================================
            # Phase 1b/1c: c_q^T, then q^T (scaled) and roped q_rot^T
            # ==============================================================
            with tc.tile_pool(name="pcq", bufs=1) as pcq, \
                 tc.tile_pool(name="p1cm", bufs=1) as p1cm, \
                 tc.tile_pool(name="p1cw", bufs=2) as p1cw:
                cqh = pcq.tile([128, c.C1P, 2, c.Q], FP8, tag="cqh")
                cql = pcq.tile([128, c.C1P, 2, c.Q], FP8, tag="cql")

                cosq_sb = p1cm.tile([128, c.Q], F32, tag="cosq")
                sinq_sb = p1cm.tile([128, c.Q], F32, tag="sinq")
                wdma(out=cosq_sb, in_=cosq[:, :])
                wdma(out=sinq_sb, in_=sinq[:, :])

                for ct in range(c.C1C):
                    csl = bass.ts(ct, 128)
                    wdqh_ct = p1bw.tile([128, c.EP, 2, 128], FP8, tag="wdqh")
                    wdma(out=wdqh_ct, in_=prr(wdqh)[:, :, :, csl])
                    wdql_ct = p1bw.tile([128, c.EP, 2, 128], FP8, tag="wdql")
                    wdma(out=wdql_ct, in_=prr(wdql)[:, :, :, csl])
                    for qt in range(c.QTN):
                        qsl = bass.ts(qt, c.QT)
                        ps = psS.tile([128, c.QT], F32, tag="s")
                        for p in range(c.EP):
                            nc.tensor.matmul(ps, wdqh_ct[:, p], xqs[qt, p][0],
                                             start=(p == 0), stop=False,
                                             perf_mode=DR)
                        for p in range(c.EP):
                            nc.tensor.matmul(ps, wdql_ct[:, p], xqs[qt, p][0],
                                             start=False, stop=False,
                                             perf_mode=DR)
                        for p in range(c.EP):
                            nc.tensor.matmul(ps, wdqh_ct[:, p], xqs[qt, p][1],
                                             start=False, stop=(p == c.EP - 1),
                                             perf_mode=DR)
                        ev_hilo(cqh[:, ct // 2, ct % 2, qsl],
                                cql[:, ct // 2, ct % 2, qsl], ps, E1,
                                bdq_sb[:, ct:ct + 1] if nzdq else None, c.QT)

                with tc.tile_pool(name="p1ct", bufs=2) as p1ct:
                    def up_chain(ps, wh_t, wl_t, qt, qsl, np_, stop_last=True):
                        for p in range(np_):
                            nc.tensor.matmul(ps, wh_t[:, p], cqh[:, p, :, qsl],
                                             start=(p == 0), stop=False,
                                             perf_mode=DR)
                        for p in range(np_):
                            nc.tensor.matmul(ps, wl_t[:, p], cqh[:, p, :, qsl],
                                             start=False, stop=False,
                                             perf_mode=DR)
                        for p in range(np_):
                            nc.tensor.matmul(ps, wh_t[:, p], cql[:, p, :, qsl],
                                             start=False,
                                             stop=(stop_last and p == np_ - 1),
                                             perf_mode=DR)

                    for h in range(c.H):
                        hsl = bass.ts(h, 128)
                        wuqh_h = p1cw.tile([128, c.C1P, 2, 128], FP8, tag="wuqh")
                        wdma(out=wuqh_h, in_=prr(wuqh)[:, :, :, hsl])
                        wuql_h = p1cw.tile([128, c.C1P, 2, 128], FP8, tag="wuql")
                        wdma(out=wuql_h, in_=prr(wuql)[:, :, :, hsl])
                        for qt in range(c.QTN):
                            qsl = bass.ts(qt, c.QT)
                            ps = psS.tile([128, c.QT], F32, tag="s")
                            up_chain(ps, wuqh_h, wuql_h, qt, qsl, c.C1P)
                            ev_hilo(qnrh[:, 0, h, qsl], qnrl[:, 0, h, qsl],
                                    ps, EQ,
                                    buq_sb[:, h:h + 1] if nzuq else None, c.QT)
                    for hp in range(c.H // 2):
                        hsl = bass.ts(hp, 128)
                        wrqh_hp = p1cw.tile([128, c.C1P, 2, 128], FP8, tag="wrqh")
                        wdma(out=wrqh_hp, in_=prr(wrqh)[:, :, :, hsl])
                        wrql_hp = p1cw.tile([128, c.C1P, 2, 128], FP8, tag="wrql")
                        wdma(out=wrql_hp, in_=prr(wrql)[:, :, :, hsl])
                        wrqrh_hp = p1cw.tile([128, c.C1P, 2, 128], FP8, tag="wrqrh")
                        wdma(out=wrqrh_hp, in_=prr(wrqrh)[:, :, :, hsl])
                        wrqrl_hp = p1cw.tile([128, c.C1P, 2, 128], FP8, tag="wrqrl")
                        wdma(out=wrqrl_hp, in_=prr(wrqrl)[:, :, :, hsl])
                        for qt in range(c.QTN):
                            qsl = bass.ts(qt, c.QT)
                            psa = psS.tile([128, c.QT], F32, tag="s")
                            up_chain(psa, wrqh_hp, wrql_hp, qt, qsl, c.C1P)
                            psar = psS.tile([128, c.QT], F32, tag="s")
                            up_chain(psar, wrqrh_hp, wrqrl_hp, qt, qsl, c.C1P)
                            tmp = p1ct.tile([128, c.QT], F32, tag="qtmp")
                            tmp2 = p1ct.tile([128, c.QT], F32, tag="qtmp2")
                            if nzrq:
                                nc.vector.scalar_tensor_tensor(
                                    tmp, psa, EQ, brq_sb[:, hp:hp + 1],
                                    ALU.mult, ALU.add)
                                nc.vector.tensor_mul(tmp, tmp, cosq_sb[:, qsl])
                                nc.vector.scalar_tensor_tensor(
                                    tmp2, psar, EQ, brqr_sb[:, hp:hp + 1],
                                    ALU.mult, ALU.add)
                                nc.vector.tensor_mul(tmp2, tmp2,
                                                     sinq_sb[:, qsl])
                            else:
                                nc.vector.scalar_tensor_tensor(
                                    tmp, psa, EQ, cosq_sb[:, qsl],
                                    ALU.mult, ALU.mult)
                                nc.vector.scalar_tensor_tensor(
                                    tmp2, psar, EQ, sinq_sb[:, qsl],
                                    ALU.mult, ALU.mult)
                            nc.vector.tensor_add(tmp, tmp, tmp2)
                            qrh = p1ct.tile([128, c.QT], FP8, tag="qrh")
                            nc.gpsimd.tensor_copy(qrh, tmp)
                            qrl = p1ct.tile([128, c.QT], FP8, tag="qrl")
                            nc.gpsimd.tensor_sub(qrl, tmp, qrh)
                            # pair-packed rows -> per-head zero-padded layout
                            gdma(out=qnrh[0:64, 1, 2 * hp, qsl], in_=qrh[0:64, :])
                            gdma(out=qnrh[0:64, 1, 2 * hp + 1, qsl],
                                 in_=qrh[64:128, :])
                            gdma(out=qnrl[0:64, 1, 2 * hp, qsl], in_=qrl[0:64, :])
                            gdma(out=qnrl[0:64, 1, 2 * hp + 1, qsl],
                                 in_=qrl[64:128, :])

        # ==================================================================
        # Phase 2: per-head attention, kT/v produced one head ahead so their
        # PSUM evictions sit before the reciprocals in engine FIFOs (v casts
        # go to the scalar engine for the same reason). Phase 3's first four
        # chains run inside head 15 to cover its tail.
        # ==================================================================
        wo_pre_h, wo_pre_l = [], []
        with tc.tile_pool(name="ow", bufs=c.H) as ow, \
             tc.tile_pool(name="oo", bufs=2) as oo, \
             tc.tile_pool(name="oa", bufs=c.QON) as oa:
            aqs = []

            def p3_chain(woh_t, wol_t, mt, qo):
                msl = bass.ts(mt, c.MT)
                aqh_t, aql_t = aqs[qo]
                ps = psS.tile([128, c.MT], F32, tag="s")
                for p in range(c.HP):
                    nc.tensor.matmul(ps, aqh_t[:, p], woh_t[p],
                                     start=(p == 0), stop=False, perf_mode=DR)
                for p in range(c.HP):
                    nc.tensor.matmul(ps, aql_t[:, p], woh_t[p],
                                     start=False, stop=False, perf_mode=DR)
                for p in range(c.HP):
                    nc.tensor.matmul(ps, aqh_t[:, p], wol_t[p],
                                     start=False,
                                     stop=(not has_bo and p == c.HP - 1),
                                     perf_mode=DR)
                if has_bo:
                    nc.tensor.matmul(ps, ones1, bo_sb[:, msl],
                                     start=False, stop=True)
                osb = oo.tile([128, c.MT], F32, tag="osb")
                nc.vector.tensor_scalar_mul(osb, ps, EO)
                adma(out=out[qo * 128:(qo + 1) * 128, msl], in_=osb)

            with tc.tile_pool(name="hk", bufs=2) as hk, \
                 tc.tile_pool(name="hv", bufs=4) as hv, \
                 tc.tile_pool(name="he", bufs=3) as he, \
                 tc.tile_pool(name="hz", bufs=2) as hz, \
                 tc.tile_pool(name="hr", bufs=2) as hr:

                v_tiles, k_tiles = {}, {}

                def produce_v(hp, pre=None):
                    if pre is None:
                        wuvh_hp = hw.tile([128, c.CP, 2, 256], FP8, tag="wuvh")
                        gdma(out=wuvh_hp,
                             in_=prr(wuvh)[:, :, :, hp * 256:(hp + 1) * 256])
                        wuvl_hp = hw.tile([128, c.CP, 2, 256], FP8, tag="wuvl")
                        gdma(out=wuvl_hp,
                             in_=prr(wuvl)[:, :, :, hp * 256:(hp + 1) * 256])
                    else:
                        wuvh_hp, wuvl_hp = pre
                    v0 = hv.tile([128, c.KC, 128], BF, tag="vh")
                    v1 = hv.tile([128, c.KC, 128], BF, tag="vh")
                    for st in range(c.KC):
                        stsl = bass.ts(st, 128)
                        psf = psA.tile([128, c.NT], F32, tag="ps")
                        ps = psf[:, 0:256]
                        for p in range(c.CP):
                            nc.tensor.matmul(ps, ckvh[:, p, :, stsl],
                                             wuvh_hp[:, p], start=(p == 0),
                                             stop=False, perf_mode=DR)
                        for p in range(c.CP):
                            nc.tensor.matmul(ps, ckvl[:, p, :, stsl],
                                             wuvh_hp[:, p], start=False,
                                             stop=False, perf_mode=DR)
                        for p in range(c.CP):
                            nc.tensor.matmul(ps, ckvh[:, p, :, stsl],
                                             wuvl_hp[:, p], start=False,
                                             stop=(not has_buv and p == c.CP - 1),
                                             perf_mode=DR)
                        if has_buv:
                            nc.tensor.matmul(ps, ones1,
                                             buv_sb[:, hp * 256:(hp + 1) * 256],
                                             start=False, stop=True)
                        nc.scalar.copy(v0[:, st, :], ps[:, 0:128])
                        nc.scalar.copy(v1[:, st, :], ps[:, 128:256])
                    v_tiles[2 * hp], v_tiles[2 * hp + 1] = v0, v1

                def produce_k(h, pre=None):
                    if pre is None:
                        wukh_h = hw.tile([128, c.CP, 2, 128], FP8, tag="wukh")
                        gdma(out=wukh_h,
                             in_=prr(wukh)[:, :, :, h * 128:(h + 1) * 128])
                        wukl_h = hw.tile([128, c.CP, 2, 128], FP8, tag="wukl")
                        gdma(out=wukl_h,
                             in_=prr(wukl)[:, :, :, h * 128:(h + 1) * 128])
                    else:
                        wukh_h, wukl_h = pre
                    knrh_t = hk.tile([128, 2, c.S], FP8, tag="knrh")
                    knrl_t = hk.tile([128, 2, c.S], FP8, tag="knrl")
                    # shared roped k_rot rides the SP ring into ktile 1
                    adma(out=knrh_t[:, 1, :], in_=krph)
                    adma(out=knrl_t[:, 1, :], in_=krpl)
                    for nt in range(c.NTN):
                        nsl = bass.ts(nt, c.NT)
                        ps = psA.tile([128, c.NT], F32, tag="ps")
                        for p in range(c.CP):
                            nc.tensor.matmul(ps, wukh_h[:, p],
                                             ckvh[:, p, :, nsl], start=(p == 0),
                                             stop=False, perf_mode=DR)
                        for p in range(c.CP):
                            nc.tensor.matmul(ps, wukl_h[:, p],
                                             ckvh[:, p, :, nsl], start=False,
                                             stop=False, perf_mode=DR)
                        for p in range(c.CP):
                            nc.tensor.matmul(ps, wukh_h[:, p],
                                             ckvl[:, p, :, nsl], start=False,
                                             stop=(p == c.CP - 1), perf_mode=DR)
                        ev_hilo(knrh_t[:, 0, nsl], knrl_t[:, 0, nsl], ps, EK,
                                buk_sb[:, h:h + 1] if nzuk else None, c.NT,
                                hi_act=True)
                    k_tiles[h] = (knrh_t, knrl_t)

                produce_v(0, pre=(wuvh_p, wuvl_p))
                produce_k(0, pre=(wukh_p0, wukl_p0))
                for h in range(c.H):
                    if h % 2 == 0 and h + 2 < c.H:
                        produce_v(h // 2 + 1)
                    if h + 1 < c.H:
                        produce_k(h + 1,
                                  pre=(wukh_p1, wukl_p1) if h == 0 else None)
                    # one wo prefetch per head for phase 3's mt=0 (hi for
                    # even heads, lo for odd - all 16 pair tiles by head 15)
                    if h % 2 == 0:
                        t = ow.tile([128, 2, c.MT], FP8, tag="wo",
                                    name=f"woh_pre{h // 2}")
                        gdma(out=t, in_=prr(woh)[:, h // 2, :, 0:c.MT])
                        wo_pre_h.append(t)
                    else:
                        t = ow.tile([128, 2, c.MT], FP8, tag="wo",
                                    name=f"wol_pre{h // 2}")
                        gdma(out=t, in_=prr(wol)[:, h // 2, :, 0:c.MT])
                        wo_pre_l.append(t)

                    knrh_t, knrl_t = k_tiles.pop(h)
                    vh = v_tiles.pop(h)
                    for qt in range(c.QTN):
                        qsl = bass.ts(qt, c.QT)
                        gps = psG.tile([128, c.QT], F32, tag="g")
                        zps = psZ.tile([128, c.QT], F32, tag="z")
                        ets, es2s = [], []
                        es2 = None
                        for kc in range(c.KC):
                            ksl = bass.ts(kc, 128)
                            sps = psS.tile([128, c.QT], F32, tag="s")
                            nc.tensor.matmul(sps, knrh_t[:, :, ksl],
                                             qnrh[:, :, h, qsl],
                                             start=True, stop=False,
                                             perf_mode=DR)
                            nc.tensor.matmul(sps, knrl_t[:, :, ksl],
                                             qnrh[:, :, h, qsl],
                                             start=False, stop=False,
                                             perf_mode=DR)
                            nc.tensor.matmul(sps, knrh_t[:, :, ksl],
                                             qnrl[:, :, h, qsl],
                                             start=False, stop=True,
                                             perf_mode=DR)
                            et = he.tile([128, c.QT], BF, tag="e")
                            nc.scalar.activation(et, sps, AF.Exp, scale=ESC)
                            nc.tensor.matmul(gps, vh[:, kc, :], et,
                                             start=(kc == 0),
                                             stop=(kc == c.KC - 1))
                            ets.append(et)
                            if kc % 2 == 1:
                                # denominator at 1/4 the PE cost: Pool
                                # pair-sums land in fp8, two pairs fill one
                                # DoubleRow ones-matmul per FOUR key chunks.
                                # Each matmul is emitted one group late so
                                # the PE FIFO never waits on a fresh sum.
                                if kc % 4 == 1:
                                    es2 = hz.tile([128, 2, c.QT], FP8,
                                                  tag="es2")
                                nc.gpsimd.tensor_add(
                                    es2[:, (kc % 4) // 2, :],
                                    ets[kc - 1], ets[kc])
                                if kc % 4 == 3:
                                    es2s.append(es2)
                                    if len(es2s) >= 2:
                                        i = len(es2s) - 2
                                        nc.tensor.matmul(
                                            zps, ones2, es2s[i],
                                            start=(i == 0), stop=False,
                                            perf_mode=DR)
                        i = len(es2s) - 1
                        nc.tensor.matmul(zps, ones2, es2s[i],
                                         start=(i == 0), stop=True,
                                         perf_mode=DR)
                        recip = hr.tile([128, c.QT], F32, tag="recip")
                        nc.vector.reciprocal(recip, zps)
                        # stage normalized attn-out in SBUF f32 so the fp8
                        # hi/lo casts can run on gpsimd (it cannot read PSUM)
                        prodf = hr.tile([128, c.QT], F32, tag="prod")
                        nc.vector.tensor_mul(prodf, gps, recip)
                        asbh = hr.tile([128, c.QT], FP8, tag="asbh")
                        nc.gpsimd.tensor_copy(asbh, prodf)
                        asbl = hr.tile([128, c.QT], FP8, tag="asbl")
                        nc.gpsimd.tensor_sub(asbl, prodf, asbh)
                        adma(out=attdh[h * 128:(h + 1) * 128, qsl], in_=asbh)
                        adma(out=attdl[h * 128:(h + 1) * 128, qsl], in_=asbl)
                        if h == c.H - 1 and qt == 0:
                            # every head's qt0 slab is in attd: prefetch
                            # phase 3's first stationaries under qt1's work
                            for qo in range(c.QON // 2):
                                qosl = bass.ts(qo, 128)
                                aqh_t = oa.tile([128, c.HP, 2, 128], FP8,
                                                tag="aqh")
                                adma(out=aqh_t, in_=prr(attdh)[:, :, :, qosl])
                                aql_t = oa.tile([128, c.HP, 2, 128], FP8,
                                                tag="aql")
                                adma(out=aql_t, in_=prr(attdl)[:, :, :, qosl])
                                aqs.append((aqh_t, aql_t))

            # ==============================================================
            # Phase 3: output projection  out[q, m] = attT.T @ wo + bo
            # ==============================================================
            for qo in range(c.QON // 2, c.QON):
                qosl = bass.ts(qo, 128)
                aqh_t = oa.tile([128, c.HP, 2, 128], FP8, tag="aqh")
                adma(out=aqh_t, in_=prr(attdh)[:, :, :, qosl])
                aql_t = oa.tile([128, c.HP, 2, 128], FP8, tag="aql")
                adma(out=aql_t, in_=prr(attdl)[:, :, :, qosl])
                aqs.append((aqh_t, aql_t))
            for mt in range(c.MTN):
                if mt == 0:
                    woh_t, wol_t = wo_pre_h, wo_pre_l
                else:
                    woh_t, wol_t = [], []
                    msl = bass.ts(mt, c.MT)
                    for p in range(c.HP):
                        t = ow.tile([128, 2, c.MT], FP8, tag="wo")
                        gdma(out=t, in_=prr(woh)[:, p, :, msl])
                        woh_t.append(t)
                    for p in range(c.HP):
                        t = ow.tile([128, 2, c.MT], FP8, tag="wo")
                        gdma(out=t, in_=prr(wol)[:, p, :, msl])
                        wol_t.append(t)
                for qo in range(c.QON):
                    p3_chain(woh_t, wol_t, mt, qo)

    return nc


# ----------------------------------------------------------------------------
# Host side: input prep, sharding, gather
# ----------------------------------------------------------------------------

def _rope_tables(seq_len, dim, theta=10000.0):
    inv_freq = 1.0 / (theta ** (np.arange(0, dim, 2, dtype=np.float32) / dim))
    t = np.arange(seq_len, dtype=np.float32)
    ang = t[:, None] * inv_freq[None, :]  # [S, dim/2]
    return np.cos(ang).astype(np.float32), np.sin(ang).astype(np.float32)


def _rot_companion_cols(w):
    """wr[..., 2i] = -w[..., 2i+1]; wr[..., 2i+1] = w[..., 2i]."""
    wr = np.empty_like(w)
    wr[..., 0::2] = -w[..., 1::2]
    wr[..., 1::2] = w[..., 0::2]
    return wr


def _hilo8(a, scale):
    """Scaled fp8 e4m3 hi/lo split: a*scale ~= hi + lo at ~2^-9 rel error."""
    import ml_dtypes
    s = np.asarray(a, np.float32) * np.float32(scale)
    hi = s.astype(ml_dtypes.float8_e4m3fn)
    lo = (s - hi.astype(np.float32)).astype(ml_dtypes.float8_e4m3fn)
    return np.ascontiguousarray(hi), np.ascontiguousarray(lo)


def host_inputs(cfg, sequence, W_dkv, b_dkv, W_dq, b_dq, W_uq, b_uq, W_uk, b_uk,
                W_uv, b_uv, W_rq, b_rq, W_rk, b_rk, W_o, b_o):
    """Build the per-core input maps for the SPMD program."""
    import ml_dtypes
    c = cfg
    f = lambda a: np.ascontiguousarray(np.asarray(a, dtype=np.float32))  # noqa: E731
    sequence = f(sequence)
    B = sequence.shape[0]
    scaler = np.float32(1.0 / np.sqrt(c.DH + c.DR))

    cos, sin = _rope_tables(c.S, c.DR)  # [S, 32]
    # rows 2i and 2i+1 both carry table column i
    cosk = np.repeat(cos.T, 2, axis=0)  # [64, S]
    sink = np.repeat(sin.T, 2, axis=0)

    wdqh, wdql = _hilo8(f(W_dq), SW1)
    wdkvh, wdkvl = _hilo8(f(W_dkv), SW1)
    wuqh, wuql = _hilo8(f(W_uq) * scaler, SWQ)
    wrqh, wrql = _hilo8(f(W_rq) * scaler, SWQ)
    wrqrh, wrqrl = _hilo8(_rot_companion_cols(f(W_rq) * scaler), SWQ)
    wrk_full = np.concatenate([f(W_rk), _rot_companion_cols(f(W_rk))], axis=1)
    wrkh, wrkl = _hilo8(wrk_full, SW1)
    wukh, wukl = _hilo8(f(W_uk), SWU)
    wuvh, wuvl = _hilo8(f(W_uv), SWU)
    woh, wol = _hilo8(f(W_o), SWO)

    shared = dict(
        wdqh=wdqh, wdql=wdql, bdq=f(b_dq) * SCKV,
        wdkvh=wdkvh, wdkvl=wdkvl, bdkv=f(b_dkv) * SCKV,
        wuqh=wuqh, wuql=wuql, buq=f(b_uq) * scaler * SQ,
        wrqh=wrqh, wrql=wrql, brq=f(b_rq) * scaler * SQ,
        wrqrh=wrqrh, wrqrl=wrqrl,
        brqr=_rot_companion_cols(f(b_rq) * scaler) * SQ,
        wrkh=wrkh, wrkl=wrkl,
        brk=np.concatenate([f(b_rk), _rot_companion_cols(f(b_rk))], axis=0) * SK,
        wukh=wukh, wukl=wukl, buk=f(b_uk) * SK,
        wuvh=wuvh, wuvl=wuvl,
        buv=(f(b_uv) * SV).astype(ml_dtypes.bfloat16),
        woh=woh, wol=wol,
        bo=(f(b_o) * SA * SWO).astype(ml_dtypes.bfloat16),
        cossink=np.concatenate([f(cosk), f(sink)], axis=0),
        ones_in=np.full((256, 128), ONEV, ml_dtypes.float8_e4m3fn),
        ones_bf=np.ones((1, 128), ml_dtypes.bfloat16),
    )
    shared = {k: np.ascontiguousarray(v) for k, v in shared.items()}

    n_cores = 2 * B
    in_maps = []
    for core in range(n_cores):
        b, half = core // 2, core % 2
        xtc = np.ascontiguousarray(sequence[b].T)         # [E, S]
        xth, xtl = _hilo8(xtc, SX)
        q0 = half * c.Q
        cq = np.tile(np.repeat(cos[q0:q0 + c.Q].T, 2, axis=0), (2, 1))  # [128, Q]
        sq = np.tile(np.repeat(sin[q0:q0 + c.Q].T, 2, axis=0), (2, 1))
        m = dict(shared)
        m.update(xth=xth, xtl=xtl,
                 xqh=np.ascontiguousarray(xth[:, q0:q0 + c.Q]),
                 xql=np.ascontiguousarray(xtl[:, q0:q0 + c.Q]),
                 cosq=np.ascontiguousarray(cq), sinq=np.ascontiguousarray(sq))
        in_maps.append(m)
    return in_maps


_PROG_CACHE = {}


def kernel(**inputs) -> np.ndarray:
    from concourse.bass_utils import run_bass_kernel_spmd

    _install_wait_split_hook()

    cfg = FULL
    nzf = lambda k: bool(np.any(np.asarray(inputs[k])))  # noqa: E731
    has_buv = nzf("b_uv")
    has_bo = nzf("b_o")
    nz = dict(bdkv=nzf("b_dkv"), bdq=nzf("b_dq"), buq=nzf("b_uq"),
              brq=nzf("b_rq"), brk=nzf("b_rk"), buk=nzf("b_uk"))
    key = ("fp8", has_buv, has_bo, tuple(sorted(nz.items())))
    if key not in _PROG_CACHE:
        _PROG_CACHE[key] = build_program(cfg, has_buv=has_buv, has_bo=has_bo,
                                         nz=nz)
    nc = _PROG_CACHE[key]

    in_maps = host_inputs(cfg, **inputs)
    n = len(in_maps)
    res = run_bass_kernel_spmd(nc, in_maps, list(range(n)))

    B = n // 2
    S = 2 * cfg.Q
    full = np.empty((B, S, cfg.DM), dtype=np.float32)
    for core in range(n):
        b, half = core // 2, core % 2
        full[b, half * cfg.Q:(half + 1) * cfg.Q, :] = res.results[core]["out"]
    return full


# revision 16
# speedup vs baseline: 1.3205x; 1.3205x over previous
"""Multi-Head Latent Attention (DeepSeek-style MLA) on 8 TRN2 NeuronCores.

Sharding: core c handles batch b = c//2 and query rows [ (c%2)*S/2, (c%2+1)*S/2 ).
Each core computes the full KV-side projections for its batch (duplicated between
the two cores sharing a batch) and the Q-side projections / attention / output
projection for its query half. No collectives; the host gathers the 8 output
shards.

Layout strategy: activations are kept feature-major ("transposed", [feature, seq])
so every matmul's contraction dim lands on SBUF partitions. Attention output is
produced directly as attT[h*128+d, q] (v as stationary operand, expT as moving),
which is exactly the lhsT layout the output projection needs - no PE transposes
anywhere. Softmax skips the max-subtraction (scores here are O(1); exp is safe)
and the denominator comes from an all-ones stationary matmul over expT.

RoPE is folded into companion weight matrices host-side:
  rope(x)[2i]   = x[2i] cos_i - x[2i+1] sin_i
  rope(x)[2i+1] = x[2i+1] cos_i + x[2i] sin_i
so with xr = x @ Wr where Wr[:,2i] = -W[:,2i+1], Wr[:,2i+1] = W[:,2i]:
  rope(x @ W) = (x @ W) * cosP + (x @ Wr) * sinP   (pure elementwise).

All matmuls run in bf16 (fp32 PSUM accumulation): same 1 cycle/row streaming
rate as float32r at N>=256, but LDWEIGHTS gets fast-weight-load (disabled for
fp32 dtypes) so the per-matmul weight swap hides under the previous matmul,
and DMA bytes / SBUF footprint halve. qT and the zero-padded per-head qrT2
stay resident in SBUF; q_rot uses a full 128-row krT stationary against
qrT2's zero pad rows (64-row stationaries measured +110ns/matmul).

Scheduling notes (each measured on HW):
- A DMA occupies its issuing engine's queue for the whole transfer, so
  traffic is spread: P1 weights + tables on the ACT ring, activation
  streams / attT / outputs on the SP ring, attention-phase weight
  prefetches and SBUF shuffles on the gpsimd SWDGE ring.
- kT/v for head h+1 are produced during head h so their PSUM evictions
  (ACT engine) sit ahead of the slow [128,512] reciprocals in engine FIFOs.
- psS has 3 banks so the scheduler can hoist the next kc's score matmul
  over the ~800ns exp latency; phase-1/3 chains share its tag, the
  produce_v/k chains take the single psA bank, att@v and the softmax
  denominator accumulate in psG/psZ (2+2).
"""

import sys
import numpy as np

sys.path.insert(0, "/opt/trn_rl_repo")

from contextlib import ExitStack  # noqa: E402

import concourse.bass as bass  # noqa: E402
import concourse.mybir as mybir  # noqa: E402
import concourse.tile as tile  # noqa: E402

F32 = mybir.dt.float32
BF = mybir.dt.bfloat16
FP8 = mybir.dt.float8e4
AF = mybir.ActivationFunctionType
ALU = mybir.AluOpType
DRW = mybir.MatmulPerfMode.DoubleRow

# Max sync-waits walrus CoreV3 codegen accepts on one instruction. The stock
# TileContext tail-drain attaches one wait per outstanding semaphore to a
# single Drain, which this walrus build rejects ("Too many sync wait
# commands"); split across several drains instead.
_MAX_WAITS_PER_INST = 1


def _split_excess_waits_json(bir_json):
    """Walrus CoreV3 codegen rejects instructions carrying more than one
    sync-wait. Tile freely attaches several. Rewrite the BIR: keep one wait on
    the instruction, move the rest onto NoOps inserted just before it on the
    same engine (a same-engine wait that fires earlier is strictly safe).
    Updates are left untouched - they must fire at instruction completion."""
    import orjson

    bir = orjson.loads(bir_json)
    n = 0
    for fn in bir.get("functions", []):
        for bb in fn.get("blocks", []):
            out = []
            for inst in bb.get("instructions", []):
                si = inst.get("sync_info")
                waits = (si or {}).get("on_wait") or []
                if len(waits) > _MAX_WAITS_PER_INST:
                    keep = waits[-_MAX_WAITS_PER_INST:]
                    for w in waits[:-_MAX_WAITS_PER_INST]:
                        out.append({
                            "name": f"I-WS{n}",
                            "opcode": "NoOp",
                            "engine": inst["engine"],
                            "ins": [],
                            "outs": [],
                            "sync_info": {"on_update": [], "on_wait": [w]},
                        })
                        n += 1
                    si["on_wait"] = keep
                out.append(inst)
            bb["instructions"] = out
    return orjson.dumps(bir)


_COMPILE_HOOKED = False


def _install_wait_split_hook():
    """Wrap compile_bir_kernel (both the bass_utils global and the name
    bass2jax imported) so every BIR headed to walrus gets the wait split."""
    global _COMPILE_HOOKED
    if _COMPILE_HOOKED:
        return
    from concourse import bass2jax, bass_utils

    orig = bass_utils.compile_bir_kernel

    def hooked(bir_json, tmpdir, neff_name="file.neff"):
        return orig(_split_excess_waits_json(bir_json), tmpdir, neff_name=neff_name)

    bass_utils.compile_bir_kernel = hooked
    bass2jax.compile_bir_kernel = hooked
    _COMPILE_HOOKED = True


class SplitDrainTileContext(tile.TileContext):
    def _drain_and_barrier(self, tick_clock, wait_clock):
        from concourse.tile_scheduler import N_PROCS
        from concourse.vector_clock import ScopedClock, VectorClock

        g = tick_clock.global_clock
        vals = [g[p] for p in range(N_PROCS)]
        nz = [p for p in range(N_PROCS) if vals[p] > 0]
        groups = [nz[i:i + _MAX_WAITS_PER_INST]
                  for i in range(0, len(nz), _MAX_WAITS_PER_INST)] or [[]]
        for grp in groups:
            sub = VectorClock([vals[p] if p in grp else 0 for p in range(N_PROCS)])
            drain_inst = self.nc.sync.drain()
            wait_clock.add_sem_waits(drain_inst.ins, ScopedClock({None: sub}))

        self.nc.all_engine_barrier()
        assert self.sems is not None
        popped = self.nc._tile_sem_poison_stack.pop()
        assert popped is self._sem_poison
        self.nc.clear_and_free_semaphores(list(self.sems.allocated().values()))
        self.nc.all_engine_barrier()


# ----------------------------------------------------------------------------
# Config
# ----------------------------------------------------------------------------

class Cfg:
    def __init__(self, E=2048, DM=2048, H=16, DC=512, DC1=1536, S=2048, Q=1024,
                 QT=512, bf16=True):
        self.E, self.DM, self.H, self.DC, self.DC1 = E, DM, H, DC, DC1
        self.S, self.Q, self.QT = S, Q, QT
        self.DR = 64          # rotary dim (fixed by the problem)
        self.DH = 128         # nope head dim (fixed: DM // H)
        self.bf16 = bf16
        assert DM == H * self.DH and H % 2 == 0
        assert E % 128 == 0 and DC % 128 == 0 and DC1 % 128 == 0
        assert S % 128 == 0
        assert Q % QT == 0 and Q % 128 == 0 and QT <= 512
        self.EC = E // 128        # embed chunks
        self.CC = DC // 128       # c_kv chunks
        self.C1C = DC1 // 128     # c_q chunks
        self.KC = S // 128        # key chunks (128-wide)
        self.ST = min(512, S)     # seq tile for phase 1
        self.STN = S // self.ST
        self.NT = min(512, S)     # kT free tile
        self.NTN = S // self.NT
        self.QTN = Q // QT
        self.MT = min(512, DM)    # out-proj free tile
        self.MTN = DM // self.MT
        self.QON = Q // 128       # out-proj q tiles


FULL = Cfg()


# ----------------------------------------------------------------------------
# Program builder (single-core SPMD program)
# ----------------------------------------------------------------------------

def build_program(cfg: Cfg, has_buv=True, has_bo=True):
    c = cfg
    FR = BF if getattr(cfg, "bf16", False) else mybir.dt.float32r
    nc = bass.Bass()
    r = lambda ap: ap  # noqa: E731

    # -- DRAM parameters -----------------------------------------------------
    xt = nc.dram_tensor("xt", [c.E, c.S], FR, kind="ExternalInput")
    xtq = nc.dram_tensor("xtq", [c.E, c.Q], FR, kind="ExternalInput")
    cosq = nc.dram_tensor("cosq", [128, c.Q], F32, kind="ExternalInput")
    sinq = nc.dram_tensor("sinq", [128, c.Q], F32, kind="ExternalInput")
    # rows 0:64 cos table, rows 64:128 sin table (packed for the fused k-rope)
    cossink = nc.dram_tensor("cossink", [128, c.S], F32, kind="ExternalInput")
    wdq = nc.dram_tensor("wdq", [c.E, c.DC1], FR, kind="ExternalInput")
    bdq = nc.dram_tensor("bdq", [c.DC1], F32, kind="ExternalInput")
    wdkv = nc.dram_tensor("wdkv", [c.E, c.DC], FR, kind="ExternalInput")
    bdkv = nc.dram_tensor("bdkv", [c.DC], F32, kind="ExternalInput")
    wuq = nc.dram_tensor("wuq", [c.DC1, c.DM], FR, kind="ExternalInput")
    buq = nc.dram_tensor("buq", [c.DM], F32, kind="ExternalInput")
    wrq = nc.dram_tensor("wrq", [c.DC1, c.H * c.DR], FR, kind="ExternalInput")
    brq = nc.dram_tensor("brq", [c.H * c.DR], F32, kind="ExternalInput")
    wrqr = nc.dram_tensor("wrqr", [c.DC1, c.H * c.DR], FR, kind="ExternalInput")
    brqr = nc.dram_tensor("brqr", [c.H * c.DR], F32, kind="ExternalInput")
    wrk = nc.dram_tensor("wrk", [c.E, 2 * c.DR], FR, kind="ExternalInput")
    brk = nc.dram_tensor("brk", [2 * c.DR], F32, kind="ExternalInput")
    wuk = nc.dram_tensor("wuk", [c.DC, c.DM], FR, kind="ExternalInput")
    buk = nc.dram_tensor("buk", [c.DM], F32, kind="ExternalInput")
    wuv = nc.dram_tensor("wuv", [c.DC, c.DM], FR, kind="ExternalInput")
    buv = nc.dram_tensor("buv", [c.DM], FR, kind="ExternalInput")
    wo = nc.dram_tensor("wo", [c.DM, c.DM], FR, kind="ExternalInput")
    bo = nc.dram_tensor("bo", [c.DM], FR, kind="ExternalInput")
    ones_d = nc.dram_tensor("ones_in", [256, 128], FP8, kind="ExternalInput")
    ones_bf_d = nc.dram_tensor("ones_bf", [1, 128], FR, kind="ExternalInput")
    out = nc.dram_tensor("out", [c.Q, c.DM], F32, kind="ExternalOutput")
    attd = nc.dram_tensor("attT_scratch", [c.DM, c.Q], BF)

    with SplitDrainTileContext(nc) as tc, ExitStack() as ctx:
        # weights / tables / small SBUF-SBUF shuffles ride the ACT HWDGE
        # ring; activation streams and output writes ride the SP ring; the
        # bulk qrT2 shuffle rides the gpsimd SWDGE ring. Rings drain in
        # parallel, so bulk weight loads never head-block the activation
        # stream (and vice versa).
        wdma = nc.scalar.dma_start
        adma = nc.sync.dma_start
        gdma = nc.gpsimd.dma_start

        # -- persistent pools ------------------------------------------------
        consts = ctx.enter_context(tc.tile_pool(name="consts", bufs=1))
        res = ctx.enter_context(tc.tile_pool(name="res", bufs=1))

        ckvT = res.tile([128, c.CC, c.S], FR, tag="ckvT")     # c_kv^T
        krT = res.tile([128, c.S], BF, tag="krT")             # roped k_rot^T, dup rows
        qT = res.tile([128, c.H, c.Q], FR, tag="qT")          # scaled q^T

        def load_pcol(name, vec, n):
            # [n*128] dram vector -> [128, n] sbuf (per-partition scalars)
            t = consts.tile([128, n], F32, tag=name)
            wdma(out=t, in_=vec.rearrange("(c p) -> p c", p=128))
            return t

        # PSUM pools (8 banks total: 2+2+2+2)
        psA = ctx.enter_context(tc.tile_pool(name="psA", bufs=1, space="PSUM"))
        psS = ctx.enter_context(tc.tile_pool(name="psS", bufs=3, space="PSUM"))
        psG = ctx.enter_context(tc.tile_pool(name="psG", bufs=2, space="PSUM"))
        psZ = ctx.enter_context(tc.tile_pool(name="psZ", bufs=2, space="PSUM"))

        paq = ctx.enter_context(tc.tile_pool(name="paq", bufs=1))
        # per-head zero-padded q_rot: rows 0:64 = head h's roped q_rot, rows
        # 64:128 = 0. Score matmuls can then use the full 128-row krT
        # stationary (the dup rows multiply zeros), which keeps LDWEIGHTS on
        # the fast path - 64-row stationaries measured +110ns per matmul.
        qrT2 = paq.tile([128, c.H, c.Q], BF, tag="qrT2")

        # head 0/1 attention weights live below the phase pools so their
        # DMAs (issued during 1c) never wait on an aliased zone
        hw = ctx.enter_context(tc.tile_pool(name="hw", bufs=3))

        # ==================================================================
        # Phase 1a: c_kv^T and roped k_rot^T over the full sequence
        # ==================================================================
        with tc.tile_pool(name="p1ax", bufs=2 * c.EC + 2) as p1ax, \
             tc.tile_pool(name="p1aw", bufs=c.EC) as p1aw, \
             tc.tile_pool(name="p1am", bufs=1) as p1am, \
             tc.tile_pool(name="p1at", bufs=4) as p1at:

            # ACT-ring issue order = need order: the 16 wdkv tiles gate the
            # first matmul chain, bdkv its PSUM eviction, wrk/cossink the
            # k_rot tail; everything else is needed phases later.
            wdkv_t, wrk_t = [], []
            for e in range(c.EC):
                wt = p1aw.tile([128, c.DC], FR, tag="wdkv", name=f"wdkv{e}")
                wdkv_t.append(wt)
            for e in range(c.EC):
                wdma(out=wdkv_t[e], in_=wdkv[e * 128:(e + 1) * 128, :])
            bdkv_sb = load_pcol("bdkv", bdkv, c.CC)
            for e in range(c.EC):
                rt = p1aw.tile([128, 2 * c.DR], FR, tag="wrk", name=f"wrk{e}")
                wrk_t.append(rt)
            for e in range(c.EC):
                wdma(out=wrk_t[e], in_=wrk[e * 128:(e + 1) * 128, :])
            brk_sb = load_pcol("brk", brk, 1)  # rows 0:64 brk, 64:128 companion
            coss_sb = p1am.tile([128, c.S], F32, tag="coss")
            wdma(out=coss_sb, in_=cossink[:, :])
            bdq_sb = load_pcol("bdq", bdq, c.C1C)
            buq_sb = load_pcol("buq", buq, c.H)
            brq_sb = load_pcol("brq", brq, c.H // 2)
            brqr_sb = load_pcol("brqr", brqr, c.H // 2)
            buk_sb = load_pcol("buk", buk, c.H)
            buv_sb = consts.tile([1, c.DM], FR, tag="buv")
            wdma(out=buv_sb, in_=buv[:].unsqueeze(0))
            bo_sb = consts.tile([1, c.DM], FR, tag="bo")
            wdma(out=bo_sb, in_=bo[:].unsqueeze(0))
            ones2 = consts.tile([128, 2, 128], FP8, tag="ones2")
            wdma(out=ones2, in_=ones_d.rearrange("(two p) m -> p two m", p=128))
            ones1 = consts.tile([1, 128], FR, tag="ones1")
            wdma(out=ones1, in_=ones_bf_d[:, :])

            for st in range(c.STN):
                ssl = bass.ts(st, c.ST)
                xts = []
                for e in range(c.EC):
                    t = p1ax.tile([128, c.ST], FR, tag="xt")
                    adma(out=t, in_=xt[e * 128:(e + 1) * 128, ssl])
                    xts.append(t)
                for ct in range(c.CC):
                    ps = psS.tile([128, c.ST], F32, tag="s")
                    for e in range(c.EC):
                        nc.tensor.matmul(ps, r(wdkv_t[e][:, ct * 128:(ct + 1) * 128]),
                                         r(xts[e]), start=(e == 0), stop=(e == c.EC - 1))
                    nc.vector.tensor_scalar_add(ckvT[:, ct, ssl], ps,
                                                bdkv_sb[:, ct:ct + 1])
                # k_rot: one 128-col stationary covers A rows (0:64, cos
                # part) and companion Ar rows (64:128, sin part) in one psum
                ps = psS.tile([128, c.ST], F32, tag="s")
                for e in range(c.EC):
                    nc.tensor.matmul(ps, r(wrk_t[e]), r(xts[e]),
                                     start=(e == 0), stop=(e == c.EC - 1))
                tmp = p1at.tile([128, c.ST], F32, tag="ktmp")
                nc.vector.scalar_tensor_tensor(tmp[0:64, :], ps[0:64, :],
                                               brk_sb[0:64, :],
                                               coss_sb[0:64, ssl], ALU.add, ALU.mult)
                nc.vector.scalar_tensor_tensor(tmp[64:128, :], ps[64:128, :],
                                               brk_sb[64:128, :],
                                               coss_sb[64:128, ssl], ALU.add, ALU.mult)
                tmp2 = p1at.tile([64, c.ST], F32, tag="ktmp2")
                gdma(out=tmp2, in_=tmp[64:128, :])
                nc.vector.tensor_add(krT[0:64, ssl], tmp[0:64, :], tmp2)
            # duplicate kr rows: rows 64:128 are the stationary rows that
            # multiply qrT2's zero rows - any finite value works, a copy is
            # the cheapest way to guarantee finite.
            gdma(out=krT[64:128, :], in_=krT[0:64, :])

        # zero qrT2's pad rows off the k_rot tail's critical path; the rot
        # matmuls (attention) are the only readers
        nc.gpsimd.memset(qrT2[64:128, :, :], 0.0)

        # pre-issue head 0/1 attention weights: the hw zone aliases nothing,
        # so these flow on the ACT ring during 1b/1c
        wuv_p = hw.tile([128, c.CC, 256], FR, tag="wuv", name="wuv_pre")
        wdma(out=wuv_p,
             in_=wuv.rearrange("(cc p) m -> p cc m", p=128)[:, :, 0:256])
        wuk_p0 = hw.tile([128, c.CC, 128], FR, tag="wuk", name="wuk_pre0")
        wdma(out=wuk_p0,
             in_=wuk.rearrange("(cc p) m -> p cc m", p=128)[:, :, 0:128])
        wuk_p1 = hw.tile([128, c.CC, 128], FR, tag="wuk", name="wuk_pre1")
        wdma(out=wuk_p1,
             in_=wuk.rearrange("(cc p) m -> p cc m", p=128)[:, :, 128:256])

        with tc.tile_pool(name="p1bx", bufs=c.QTN * c.EC + 2) as p1bx, \
             tc.tile_pool(name="p1bw", bufs=2) as p1bw:
            # 1b's activations: fresh zone, so these queue dep-free on the
            # SP ring right behind 1a's xt stream
            xqs = {}
            for qt in range(c.QTN):
                qsl = bass.ts(qt, c.QT)
                for e in range(c.EC):
                    t = p1bx.tile([128, c.QT], FR, tag="xq")
                    adma(out=t, in_=xtq[e * 128:(e + 1) * 128, qsl])
                    xqs[qt, e] = t

            # ==============================================================
            # Phase 1b/1c: c_q^T, then q^T (scaled) and roped q_rot^T
            # ==============================================================
            with tc.tile_pool(name="pcq", bufs=1) as pcq, \
                 tc.tile_pool(name="p1cm", bufs=1) as p1cm, \
                 tc.tile_pool(name="p1cw", bufs=2) as p1cw:
                cqT = pcq.tile([128, c.C1C, c.Q], FR, tag="cqT")

                cosq_sb = p1cm.tile([128, c.Q], F32, tag="cosq")
                sinq_sb = p1cm.tile([128, c.Q], F32, tag="sinq")
                wdma(out=cosq_sb, in_=cosq[:, :])
                wdma(out=sinq_sb, in_=sinq[:, :])

                for ct in range(c.C1C):
                    wdq_ct = p1bw.tile([128, c.EC, 128], FR, tag="wdq")
                    wdma(
                        out=wdq_ct,
                        in_=wdq.rearrange("(e p) m -> p e m", p=128)[:, :, ct * 128:(ct + 1) * 128])
                    for qt in range(c.QTN):
                        qsl = bass.ts(qt, c.QT)
                        ps = psS.tile([128, c.QT], F32, tag="s")
                        for e in range(c.EC):
                            nc.tensor.matmul(ps, r(wdq_ct[:, e, :]), r(xqs[qt, e]),
                                             start=(e == 0), stop=(e == c.EC - 1))
                        nc.vector.tensor_scalar_add(cqT[:, ct, qsl], ps,
                                                    bdq_sb[:, ct:ct + 1])

                with tc.tile_pool(name="p1ct", bufs=4) as p1ct:
                    for h in range(c.H):
                        wuq_h = p1cw.tile([128, c.C1C, 128], FR, tag="wuq")
                        wdma(
                            out=wuq_h,
                            in_=wuq.rearrange("(cc p) m -> p cc m", p=128)[:, :, h * 128:(h + 1) * 128])
                        for qt in range(c.QTN):
                            qsl = bass.ts(qt, c.QT)
                            ps = psS.tile([128, c.QT], F32, tag="s")
                            for ct in range(c.C1C):
                                nc.tensor.matmul(ps, r(wuq_h[:, ct, :]), r(cqT[:, ct, qsl]),
                                                 start=(ct == 0), stop=(ct == c.C1C - 1))
                            nc.vector.tensor_scalar_add(qT[:, h, qsl], ps,
                                                        buq_sb[:, h:h + 1])
                    for hp in range(c.H // 2):
                        wrq_hp = p1cw.tile([128, c.C1C, 128], FR, tag="wrq")
                        wdma(
                            out=wrq_hp,
                            in_=wrq.rearrange("(cc p) m -> p cc m", p=128)[:, :, hp * 128:(hp + 1) * 128])
                        wrqr_hp = p1cw.tile([128, c.C1C, 128], FR, tag="wrqr")
                        wdma(
                            out=wrqr_hp,
                            in_=wrqr.rearrange("(cc p) m -> p cc m", p=128)[:, :, hp * 128:(hp + 1) * 128])
                        for qt in range(c.QTN):
                            qsl = bass.ts(qt, c.QT)
                            psa = psS.tile([128, c.QT], F32, tag="s")
                            for ct in range(c.C1C):
                                nc.tensor.matmul(psa, r(wrq_hp[:, ct, :]), r(cqT[:, ct, qsl]),
                                                 start=(ct == 0), stop=(ct == c.C1C - 1))
                            psar = psS.tile([128, c.QT], F32, tag="s")
                            for ct in range(c.C1C):
                                nc.tensor.matmul(psar, r(wrqr_hp[:, ct, :]), r(cqT[:, ct, qsl]),
                                                 start=(ct == 0), stop=(ct == c.C1C - 1))
                            tmp = p1ct.tile([128, c.QT], F32, tag="qtmp")
                            nc.vector.scalar_tensor_tensor(tmp, psa, brq_sb[:, hp:hp + 1],
                                                           cosq_sb[:, qsl], ALU.add, ALU.mult)
                            qrp = p1ct.tile([128, c.QT], BF, tag="qrp")
                            nc.vector.scalar_tensor_tensor(qrp, psar,
                                                           brqr_sb[:, hp:hp + 1],
                                                           sinq_sb[:, qsl], ALU.add, ALU.mult)
                            nc.vector.tensor_add(qrp, qrp, tmp)
                            # pair-packed rows -> per-head zero-padded layout
                            gdma(out=qrT2[0:64, 2 * hp, qsl], in_=qrp[0:64, :])
                            gdma(out=qrT2[0:64, 2 * hp + 1, qsl], in_=qrp[64:128, :])

        # ==================================================================
        # Phase 2: per-head attention, kT/v produced one head ahead so their
        # PSUM evictions sit before the reciprocals in engine FIFOs (v casts
        # go to the scalar engine for the same reason). Phase 3's first four
        # chains run inside head 15 to cover its tail.
        # ==================================================================
        wo_pre = []
        with tc.tile_pool(name="ow", bufs=c.H + 2) as ow, \
             tc.tile_pool(name="oo", bufs=2) as oo, \
             tc.tile_pool(name="oa", bufs=c.QON) as oa:
            aqs = []

            def p3_chain(wo_t, mt, qo):
                msl = bass.ts(mt, c.MT)
                ps = psS.tile([128, c.MT], F32, tag="s")
                for hc in range(c.H):
                    nc.tensor.matmul(ps, r(aqs[qo][:, hc, :]), r(wo_t[hc]),
                                     start=(hc == 0),
                                     stop=(not has_bo and hc == c.H - 1))
                if has_bo:
                    nc.tensor.matmul(ps, r(ones1), r(bo_sb[:, msl]),
                                     start=False, stop=True)
                osb = oo.tile([128, c.MT], F32, tag="osb")
                nc.vector.tensor_copy(osb, ps)
                adma(out=out[qo * 128:(qo + 1) * 128, msl], in_=osb)

            with tc.tile_pool(name="hk", bufs=2) as hk, \
                 tc.tile_pool(name="hv", bufs=4) as hv, \
                 tc.tile_pool(name="he", bufs=4) as he, \
                 tc.tile_pool(name="hz", bufs=2) as hz, \
                 tc.tile_pool(name="hr", bufs=2) as hr:

                v_tiles, k_tiles = {}, {}

                def produce_v(hp, pre=None):
                    if pre is None:
                        wuv_hp = hw.tile([128, c.CC, 256], FR, tag="wuv")
                        gdma(
                            out=wuv_hp,
                            in_=wuv.rearrange("(cc p) m -> p cc m", p=128)[:, :, hp * 256:(hp + 1) * 256])
                    else:
                        wuv_hp = pre
                    v0 = hv.tile([128, c.KC, 128], FR, tag="vh")
                    v1 = hv.tile([128, c.KC, 128], FR, tag="vh")
                    for st in range(c.KC):
                        psf = psA.tile([128, c.NT], F32, tag="ps")
                        ps = psf[:, 0:256]
                        for cc in range(c.CC):
                            nc.tensor.matmul(ps, r(ckvT[:, cc, st * 128:(st + 1) * 128]),
                                             r(wuv_hp[:, cc, :]),
                                             start=(cc == 0),
                                             stop=(not has_buv and cc == c.CC - 1))
                        if has_buv:
                            nc.tensor.matmul(ps, r(ones1),
                                             r(buv_sb[:, hp * 256:(hp + 1) * 256]),
                                             start=False, stop=True)
                        nc.scalar.copy(v0[:, st, :], ps[:, 0:128])
                        nc.scalar.copy(v1[:, st, :], ps[:, 128:256])
                    v_tiles[2 * hp], v_tiles[2 * hp + 1] = v0, v1

                def produce_k(h, pre=None):
                    if pre is None:
                        wuk_h = hw.tile([128, c.CC, 128], FR, tag="wuk")
                        gdma(
                            out=wuk_h,
                            in_=wuk.rearrange("(cc p) m -> p cc m", p=128)[:, :, h * 128:(h + 1) * 128])
                    else:
                        wuk_h = pre
                    kT = hk.tile([128, c.S], FR, tag="kT")
                    for nt in range(c.NTN):
                        nsl = bass.ts(nt, c.NT)
                        ps = psA.tile([128, c.NT], F32, tag="ps")
                        for cc in range(c.CC):
                            nc.tensor.matmul(ps, r(wuk_h[:, cc, :]), r(ckvT[:, cc, nsl]),
                                             start=(cc == 0), stop=(cc == c.CC - 1))
                        nc.scalar.add(kT[:, nsl], ps, buk_sb[:, h:h + 1])
                    k_tiles[h] = kT

                produce_v(0, pre=wuv_p)
                produce_k(0, pre=wuk_p0)
                for h in range(c.H):
                    if h % 2 == 0 and h + 2 < c.H:
                        produce_v(h // 2 + 1)
                    if h + 1 < c.H:
                        produce_k(h + 1, pre=wuk_p1 if h == 0 else None)
                    # one wo prefetch per head for phase 3's mt=0
                    t = ow.tile([128, c.MT], FR, tag="wo")
                    gdma(out=t, in_=wo[h * 128:(h + 1) * 128, 0:c.MT])
                    wo_pre.append(t)

                    kT = k_tiles.pop(h)
                    vh = v_tiles.pop(h)
                    for qt in range(c.QTN):
                        qsl = bass.ts(qt, c.QT)
                        gps = psG.tile([128, c.QT], F32, tag="g")
                        zps = psZ.tile([128, c.QT], F32, tag="z")
                        ets, ess, es2s = [], [], []
                        esf = hz.tile([128, 2, c.QT], FP8, tag="esf")
                        for kc in range(c.KC):
                            ksl = bass.ts(kc, 128)
                            sps = psS.tile([128, c.QT], F32, tag="s")
                            nc.tensor.matmul(sps, r(kT[:, ksl]), r(qT[:, h, qsl]),
                                             start=True, stop=False)
                            nc.tensor.matmul(sps, r(krT[:, ksl]),
                                             r(qrT2[:, h, qsl]),
                                             start=False, stop=True)
                            et = he.tile([128, c.QT], FR, tag="e")
                            nc.scalar.activation(et, sps, AF.Exp)
                            nc.tensor.matmul(gps, r(vh[:, kc, :]), r(et),
                                             start=(kc == 0), stop=(kc == c.KC - 1))
                            ets.append(et)
                            # denominator at 1/8 the PE cost: a 3-level
                            # pair-sum tree (Pool, DVE, Pool) collapses the 16
                            # exp tiles into two fp8 partials that one fp8
                            # DoubleRow ones-matmul reduces in a single pass.
                            if kc % 2 == 1:
                                es = hz.tile([128, c.QT], FR, tag="es")
                                nc.gpsimd.tensor_add(es, ets[kc - 1], ets[kc])
                                ess.append(es)
                            if kc % 4 == 3:
                                es2 = hz.tile([128, c.QT], FR, tag="es2")
                                nc.vector.tensor_add(es2, ess[-2], ess[-1])
                                es2s.append(es2)
                            if kc % 8 == 7:
                                nc.gpsimd.tensor_add(esf[:, (kc // 8), :],
                                                     es2s[-2], es2s[-1])
                        nc.tensor.matmul(zps, ones2, esf, start=True, stop=True,
                                         perf_mode=DRW)
                        recip = hr.tile([128, c.QT], F32, tag="recip")
                        nc.vector.reciprocal(recip, zps)
                        asb = hr.tile([128, c.QT], BF, tag="attsb")
                        nc.vector.tensor_mul(asb, gps, recip)
                        adma(out=attd[h * 128:(h + 1) * 128, qsl], in_=asb)
                        if h == c.H - 1 and qt == 0:
                            # every head's qt0 slab is in attd: prefetch
                            # phase 3's first stationaries under qt1's work
                            for qo in range(c.QON // 2):
                                aq = oa.tile([128, c.H, 128], BF, tag="attq")
                                adma(
                                    out=aq,
                                    in_=attd.rearrange("(hc p) q -> p hc q", p=128)[:, :, qo * 128:(qo + 1) * 128])
                                aqs.append(aq)

            # ==============================================================
            # Phase 3: output projection  out[q, m] = attT.T @ wo + bo
            # ==============================================================
            for qo in range(c.QON // 2, c.QON):
                aq = oa.tile([128, c.H, 128], BF, tag="attq")
                adma(
                    out=aq,
                    in_=attd.rearrange("(hc p) q -> p hc q", p=128)[:, :, qo * 128:(qo + 1) * 128])
                aqs.append(aq)
            for mt in range(c.MTN):
                if mt == 0:
                    wo_t = wo_pre
                    qos = range(c.QON)
                else:
                    wo_t = []
                    for hc in range(c.H):
                        t = ow.tile([128, c.MT], FR, tag="wo")
                        gdma(out=t, in_=wo[hc * 128:(hc + 1) * 128, bass.ts(mt, c.MT)])
                        wo_t.append(t)
                    qos = range(c.QON)
                for qo in qos:
                    p3_chain(wo_t, mt, qo)

    return nc


# ----------------------------------------------------------------------------
# Host side: input prep, sharding, gather
# ----------------------------------------------------------------------------

def _rope_tables(seq_len, dim, theta=10000.0):
    inv_freq = 1.0 / (theta ** (np.arange(0, dim, 2, dtype=np.float32) / dim))
    t = np.arange(seq_len, dtype=np.float32)
    ang = t[:, None] * inv_freq[None, :]  # [S, dim/2]
    return np.cos(ang).astype(np.float32), np.sin(ang).astype(np.float32)


def _rot_companion_cols(w):
    """wr[..., 2i] = -w[..., 2i+1]; wr[..., 2i+1] = w[..., 2i]."""
    wr = np.empty_like(w)
    wr[..., 0::2] = -w[..., 1::2]
    wr[..., 1::2] = w[..., 0::2]
    return wr


def host_inputs(cfg, sequence, W_dkv, b_dkv, W_dq, b_dq, W_uq, b_uq, W_uk, b_uk,
                W_uv, b_uv, W_rq, b_rq, W_rk, b_rk, W_o, b_o):
    """Build the per-core input maps for the SPMD program."""
    c = cfg
    f = lambda a: np.ascontiguousarray(np.asarray(a, dtype=np.float32))  # noqa: E731
    sequence = f(sequence)
    B = sequence.shape[0]
    scaler = np.float32(1.0 / np.sqrt(c.DH + c.DR))

    cos, sin = _rope_tables(c.S, c.DR)  # [S, 32]
    # rows 2i and 2i+1 both carry table column i
    cosk = np.repeat(cos.T, 2, axis=0)  # [64, S]
    sink = np.repeat(sin.T, 2, axis=0)

    shared = dict(
        wdq=f(W_dq), bdq=f(b_dq),
        wdkv=f(W_dkv), bdkv=f(b_dkv),
        wuq=f(W_uq) * scaler, buq=f(b_uq) * scaler,
        wrq=f(W_rq) * scaler, brq=f(b_rq) * scaler,
        wrqr=_rot_companion_cols(f(W_rq) * scaler),
        brqr=_rot_companion_cols(f(b_rq) * scaler),
        wrk=np.concatenate([f(W_rk), _rot_companion_cols(f(W_rk))], axis=1),
        brk=np.concatenate([f(b_rk), _rot_companion_cols(f(b_rk))], axis=0),
        wuk=f(W_uk), buk=f(b_uk),
        wuv=f(W_uv), buv=f(b_uv),
        wo=f(W_o), bo=f(b_o),
        cossink=np.concatenate([f(cosk), f(sink)], axis=0),
        ones_in=np.ones((256, 128), np.float32),
        ones_bf=np.ones((1, 128), np.float32),
    )
    shared = {k: np.ascontiguousarray(v) for k, v in shared.items()}
    mm_keys = {"wdq", "wdkv", "wuq", "wrq", "wrqr", "wrk", "wuk", "wuv", "wo",
               "buv", "bo", "ones_bf"}
    if getattr(c, "bf16", False):
        import ml_dtypes
        for k in mm_keys:
            shared[k] = shared[k].astype(ml_dtypes.bfloat16)
    import ml_dtypes
    shared["ones_in"] = shared["ones_in"].astype(ml_dtypes.float8_e4m3fn)

    n_cores = 2 * B
    in_maps = []
    for core in range(n_cores):
        b, half = core // 2, core % 2
        xtc = np.ascontiguousarray(sequence[b].T)         # [E, S]
        q0 = half * c.Q
        xtqc = np.ascontiguousarray(xtc[:, q0:q0 + c.Q])  # [E, Q]
        cq = np.tile(np.repeat(cos[q0:q0 + c.Q].T, 2, axis=0), (2, 1))  # [128, Q]
        sq = np.tile(np.repeat(sin[q0:q0 + c.Q].T, 2, axis=0), (2, 1))
        m = dict(shared)
        if getattr(c, "bf16", False):
            import ml_dtypes
            xtc = xtc.astype(ml_dtypes.bfloat16)
            xtqc = xtqc.astype(ml_dtypes.bfloat16)
        m.update(xt=xtc, xtq=xtqc,
                 cosq=np.ascontiguousarray(cq), sinq=np.ascontiguousarray(sq))
        in_maps.append(m)
    return in_maps


_PROG_CACHE = {}


def kernel(**inputs) -> np.ndarray:
    from concourse.bass_utils import run_bass_kernel_spmd

    _install_wait_split_hook()

    cfg = FULL
    has_buv = bool(np.any(np.asarray(inputs["b_uv"])))
    has_bo = bool(np.any(np.asarray(inputs["b_o"])))
    key = ("full", has_buv, has_bo)
    if key not in _PROG_CACHE:
        _PROG_CACHE[key] = build_program(cfg, has_buv=has_buv, has_bo=has_bo)
    nc = _PROG_CACHE[key]

    in_maps = host_inputs(cfg, **inputs)
    n = len(in_maps)
    res = run_bass_kernel_spmd(nc, in_maps, list(range(n)))

    B = n // 2
    S = 2 * cfg.Q
    full = np.empty((B, S, cfg.DM), dtype=np.float32)
    for core in range(n):
        b, half = core // 2, core % 2
        full[b, half * cfg.Q:(half + 1) * cfg.Q, :] = res.results[core]["out"]
    return full



# revision 17
# speedup vs baseline: 1.4414x; 1.0916x over previous
"""Multi-Head Latent Attention (DeepSeek-style MLA) on 8 TRN2 NeuronCores.

Sharding: core c handles batch b = c//2 and query rows [ (c%2)*S/2, (c%2+1)*S/2 ).
Each core computes the full KV-side projections for its batch (duplicated between
the two cores sharing a batch) and the Q-side projections / attention / output
projection for its query half. No collectives; the host gathers the 8 output
shards.

Layout strategy: activations are kept feature-major ("transposed", [feature, seq])
so every matmul's contraction dim lands on SBUF partitions. Attention output is
produced directly as attT[h*128+d, q] (v as stationary operand, expT as moving),
which is exactly the lhsT layout the output projection needs - no PE transposes
anywhere. Softmax skips the max-subtraction (scores here are O(1); exp is safe)
and the denominator comes from an all-ones stationary matmul over expT.

RoPE is folded into companion weight matrices host-side:
  rope(x)[2i]   = x[2i] cos_i - x[2i+1] sin_i
  rope(x)[2i+1] = x[2i+1] cos_i + x[2i] sin_i
so with xr = x @ Wr where Wr[:,2i] = -W[:,2i+1], Wr[:,2i+1] = W[:,2i]:
  rope(x @ W) = (x @ W) * cosP + (x @ Wr) * sinP   (pure elementwise).

All matmuls run in bf16 (fp32 PSUM accumulation): same 1 cycle/row streaming
rate as float32r at N>=256, but LDWEIGHTS gets fast-weight-load (disabled for
fp32 dtypes) so the per-matmul weight swap hides under the previous matmul,
and DMA bytes / SBUF footprint halve. qT and the zero-padded per-head qrT2
stay resident in SBUF; q_rot uses a full 128-row krT stationary against
qrT2's zero pad rows (64-row stationaries measured +110ns/matmul).

Scheduling notes (each measured on HW):
- A DMA occupies its issuing engine's queue for the whole transfer, so
  traffic is spread: P1 weights + tables on the ACT ring, activation
  streams / attT / outputs on the SP ring, attention-phase weight
  prefetches and SBUF shuffles on the gpsimd SWDGE ring.
- kT/v for head h+1 are produced during head h so their PSUM evictions
  (ACT engine) sit ahead of the slow [128,512] reciprocals in engine FIFOs.
- psS has 3 banks so the scheduler can hoist the next kc's score matmul
  over the ~800ns exp latency; phase-1/3 chains share its tag, the
  produce_v/k chains take the single psA bank, att@v and the softmax
  denominator accumulate in psG/psZ (2+2).
"""

import sys
import numpy as np

sys.path.insert(0, "/opt/trn_rl_repo")

from contextlib import ExitStack  # noqa: E402

import concourse.bass as bass  # noqa: E402
import concourse.mybir as mybir  # noqa: E402
import concourse.tile as tile  # noqa: E402

F32 = mybir.dt.float32
BF = mybir.dt.bfloat16
FP8 = mybir.dt.float8e4
AF = mybir.ActivationFunctionType
ALU = mybir.AluOpType
DRW = mybir.MatmulPerfMode.DoubleRow

# Max sync-waits walrus CoreV3 codegen accepts on one instruction. The stock
# TileContext tail-drain attaches one wait per outstanding semaphore to a
# single Drain, which this walrus build rejects ("Too many sync wait
# commands"); split across several drains instead.
_MAX_WAITS_PER_INST = 1


def _split_excess_waits_json(bir_json):
    """Walrus CoreV3 codegen rejects instructions carrying more than one
    sync-wait. Tile freely attaches several. Rewrite the BIR: keep one wait on
    the instruction, move the rest onto NoOps inserted just before it on the
    same engine (a same-engine wait that fires earlier is strictly safe).
    Updates are left untouched - they must fire at instruction completion."""
    import orjson

    bir = orjson.loads(bir_json)
    n = 0
    for fn in bir.get("functions", []):
        for bb in fn.get("blocks", []):
            out = []
            for inst in bb.get("instructions", []):
                si = inst.get("sync_info")
                waits = (si or {}).get("on_wait") or []
                if len(waits) > _MAX_WAITS_PER_INST:
                    keep = waits[-_MAX_WAITS_PER_INST:]
                    for w in waits[:-_MAX_WAITS_PER_INST]:
                        out.append({
                            "name": f"I-WS{n}",
                            "opcode": "NoOp",
                            "engine": inst["engine"],
                            "ins": [],
                            "outs": [],
                            "sync_info": {"on_update": [], "on_wait": [w]},
                        })
                        n += 1
                    si["on_wait"] = keep
                out.append(inst)
            bb["instructions"] = out
    return orjson.dumps(bir)


_COMPILE_HOOKED = False


def _install_wait_split_hook():
    """Wrap compile_bir_kernel (both the bass_utils global and the name
    bass2jax imported) so every BIR headed to walrus gets the wait split."""
    global _COMPILE_HOOKED
    if _COMPILE_HOOKED:
        return
    from concourse import bass2jax, bass_utils

    orig = bass_utils.compile_bir_kernel

    def hooked(bir_json, tmpdir, neff_name="file.neff"):
        return orig(_split_excess_waits_json(bir_json), tmpdir, neff_name=neff_name)

    bass_utils.compile_bir_kernel = hooked
    bass2jax.compile_bir_kernel = hooked
    _COMPILE_HOOKED = True


class SplitDrainTileContext(tile.TileContext):
    def _drain_and_barrier(self, tick_clock, wait_clock):
        from concourse.tile_scheduler import N_PROCS
        from concourse.vector_clock import ScopedClock, VectorClock

        g = tick_clock.global_clock
        vals = [g[p] for p in range(N_PROCS)]
        nz = [p for p in range(N_PROCS) if vals[p] > 0]
        groups = [nz[i:i + _MAX_WAITS_PER_INST]
                  for i in range(0, len(nz), _MAX_WAITS_PER_INST)] or [[]]
        for grp in groups:
            sub = VectorClock([vals[p] if p in grp else 0 for p in range(N_PROCS)])
            drain_inst = self.nc.sync.drain()
            wait_clock.add_sem_waits(drain_inst.ins, ScopedClock({None: sub}))

        self.nc.all_engine_barrier()
        assert self.sems is not None
        popped = self.nc._tile_sem_poison_stack.pop()
        assert popped is self._sem_poison
        self.nc.clear_and_free_semaphores(list(self.sems.allocated().values()))
        self.nc.all_engine_barrier()


# ----------------------------------------------------------------------------
# Config
# ----------------------------------------------------------------------------

class Cfg:
    def __init__(self, E=2048, DM=2048, H=16, DC=512, DC1=1536, S=2048, Q=1024,
                 QT=512, bf16=True):
        self.E, self.DM, self.H, self.DC, self.DC1 = E, DM, H, DC, DC1
        self.S, self.Q, self.QT = S, Q, QT
        self.DR = 64          # rotary dim (fixed by the problem)
        self.DH = 128         # nope head dim (fixed: DM // H)
        self.bf16 = bf16
        assert DM == H * self.DH and H % 2 == 0
        assert E % 128 == 0 and DC % 128 == 0 and DC1 % 128 == 0
        assert S % 128 == 0
        assert Q % QT == 0 and Q % 128 == 0 and QT <= 512
        self.EC = E // 128        # embed chunks
        self.CC = DC // 128       # c_kv chunks
        self.C1C = DC1 // 128     # c_q chunks
        self.KC = S // 128        # key chunks (128-wide)
        self.ST = min(512, S)     # seq tile for phase 1
        self.STN = S // self.ST
        self.NT = min(512, S)     # kT free tile
        self.NTN = S // self.NT
        self.QTN = Q // QT
        self.MT = min(512, DM)    # out-proj free tile
        self.MTN = DM // self.MT
        self.QON = Q // 128       # out-proj q tiles


FULL = Cfg()


# ----------------------------------------------------------------------------
# Program builder (single-core SPMD program)
# ----------------------------------------------------------------------------

def build_program(cfg: Cfg, has_buv=True, has_bo=True):
    c = cfg
    FR = BF if getattr(cfg, "bf16", False) else mybir.dt.float32r
    nc = bass.Bass()
    r = lambda ap: ap  # noqa: E731

    # -- DRAM parameters -----------------------------------------------------
    xt = nc.dram_tensor("xt", [c.E, c.S], FR, kind="ExternalInput")
    xtq = nc.dram_tensor("xtq", [c.E, c.Q], FR, kind="ExternalInput")
    cosq = nc.dram_tensor("cosq", [128, c.Q], F32, kind="ExternalInput")
    sinq = nc.dram_tensor("sinq", [128, c.Q], F32, kind="ExternalInput")
    # rows 0:64 cos table, rows 64:128 sin table (packed for the fused k-rope)
    cossink = nc.dram_tensor("cossink", [128, c.S], F32, kind="ExternalInput")
    wdq = nc.dram_tensor("wdq", [c.E, c.DC1], FR, kind="ExternalInput")
    bdq = nc.dram_tensor("bdq", [c.DC1], F32, kind="ExternalInput")
    wdkv = nc.dram_tensor("wdkv", [c.E, c.DC], FR, kind="ExternalInput")
    bdkv = nc.dram_tensor("bdkv", [c.DC], F32, kind="ExternalInput")
    wuq = nc.dram_tensor("wuq", [c.DC1, c.DM], FR, kind="ExternalInput")
    buq = nc.dram_tensor("buq", [c.DM], F32, kind="ExternalInput")
    wrq = nc.dram_tensor("wrq", [c.DC1, c.H * c.DR], FR, kind="ExternalInput")
    brq = nc.dram_tensor("brq", [c.H * c.DR], F32, kind="ExternalInput")
    wrqr = nc.dram_tensor("wrqr", [c.DC1, c.H * c.DR], FR, kind="ExternalInput")
    brqr = nc.dram_tensor("brqr", [c.H * c.DR], F32, kind="ExternalInput")
    wrk = nc.dram_tensor("wrk", [c.E, 2 * c.DR], FR, kind="ExternalInput")
    brk = nc.dram_tensor("brk", [2 * c.DR], F32, kind="ExternalInput")
    wuk = nc.dram_tensor("wuk", [c.DC, c.DM], FR, kind="ExternalInput")
    buk = nc.dram_tensor("buk", [c.DM], F32, kind="ExternalInput")
    wuv = nc.dram_tensor("wuv", [c.DC, c.DM], FR, kind="ExternalInput")
    buv = nc.dram_tensor("buv", [c.DM], FR, kind="ExternalInput")
    wo = nc.dram_tensor("wo", [c.DM, c.DM], FR, kind="ExternalInput")
    bo = nc.dram_tensor("bo", [c.DM], FR, kind="ExternalInput")
    ones_d = nc.dram_tensor("ones_in", [256, 128], FP8, kind="ExternalInput")
    ones_bf_d = nc.dram_tensor("ones_bf", [1, 128], FR, kind="ExternalInput")
    out = nc.dram_tensor("out", [c.Q, c.DM], F32, kind="ExternalOutput")
    attd = nc.dram_tensor("attT_scratch", [c.DM, c.Q], BF)

    with SplitDrainTileContext(nc) as tc, ExitStack() as ctx:
        # weights / tables / small SBUF-SBUF shuffles ride the ACT HWDGE
        # ring; activation streams and output writes ride the SP ring; the
        # bulk qrT2 shuffle rides the gpsimd SWDGE ring. Rings drain in
        # parallel, so bulk weight loads never head-block the activation
        # stream (and vice versa).
        wdma = nc.scalar.dma_start
        adma = nc.sync.dma_start
        gdma = nc.gpsimd.dma_start

        # -- persistent pools ------------------------------------------------
        consts = ctx.enter_context(tc.tile_pool(name="consts", bufs=1))
        res = ctx.enter_context(tc.tile_pool(name="res", bufs=1))

        ckvT = res.tile([128, c.CC, c.S], FR, tag="ckvT")     # c_kv^T
        krT = res.tile([128, c.S], BF, tag="krT")             # roped k_rot^T, dup rows
        qT = res.tile([128, c.H, c.Q], FR, tag="qT")          # scaled q^T

        def load_pcol(name, vec, n):
            # [n*128] dram vector -> [128, n] sbuf (per-partition scalars)
            t = consts.tile([128, n], F32, tag=name)
            wdma(out=t, in_=vec.rearrange("(c p) -> p c", p=128))
            return t

        # PSUM pools (8 banks total: 2+2+2+2)
        psA = ctx.enter_context(tc.tile_pool(name="psA", bufs=1, space="PSUM"))
        psS = ctx.enter_context(tc.tile_pool(name="psS", bufs=3, space="PSUM"))
        psG = ctx.enter_context(tc.tile_pool(name="psG", bufs=2, space="PSUM"))
        psZ = ctx.enter_context(tc.tile_pool(name="psZ", bufs=2, space="PSUM"))

        paq = ctx.enter_context(tc.tile_pool(name="paq", bufs=1))
        # per-head zero-padded q_rot: rows 0:64 = head h's roped q_rot, rows
        # 64:128 = 0. Score matmuls can then use the full 128-row krT
        # stationary (the dup rows multiply zeros), which keeps LDWEIGHTS on
        # the fast path - 64-row stationaries measured +110ns per matmul.
        qrT2 = paq.tile([128, c.H, c.Q], BF, tag="qrT2")

        # head 0/1 attention weights live below the phase pools so their
        # DMAs (issued during 1c) never wait on an aliased zone
        hw = ctx.enter_context(tc.tile_pool(name="hw", bufs=3))

        # ==================================================================
        # Phase 1a: c_kv^T and roped k_rot^T over the full sequence
        # ==================================================================
        with tc.tile_pool(name="p1ax", bufs=2 * c.EC + 2) as p1ax, \
             tc.tile_pool(name="p1aw", bufs=c.EC) as p1aw, \
             tc.tile_pool(name="p1am", bufs=1) as p1am, \
             tc.tile_pool(name="p1at", bufs=4) as p1at:

            # ACT-ring issue order = need order: the 16 wdkv tiles gate the
            # first matmul chain, bdkv its PSUM eviction, wrk/cossink the
            # k_rot tail; everything else is needed phases later.
            wdkv_t, wrk_t = [], []
            for e in range(c.EC):
                wt = p1aw.tile([128, c.DC], FR, tag="wdkv", name=f"wdkv{e}")
                wdkv_t.append(wt)
            for e in range(c.EC):
                wdma(out=wdkv_t[e], in_=wdkv[e * 128:(e + 1) * 128, :])
            bdkv_sb = load_pcol("bdkv", bdkv, c.CC)
            for e in range(c.EC):
                rt = p1aw.tile([128, 2 * c.DR], FR, tag="wrk", name=f"wrk{e}")
                wrk_t.append(rt)
            for e in range(c.EC):
                wdma(out=wrk_t[e], in_=wrk[e * 128:(e + 1) * 128, :])
            brk_sb = load_pcol("brk", brk, 1)  # rows 0:64 brk, 64:128 companion
            coss_sb = p1am.tile([128, c.S], F32, tag="coss")
            wdma(out=coss_sb, in_=cossink[:, :])
            bdq_sb = load_pcol("bdq", bdq, c.C1C)
            buq_sb = load_pcol("buq", buq, c.H)
            brq_sb = load_pcol("brq", brq, c.H // 2)
            brqr_sb = load_pcol("brqr", brqr, c.H // 2)
            buk_sb = load_pcol("buk", buk, c.H)
            buv_sb = consts.tile([1, c.DM], FR, tag="buv")
            wdma(out=buv_sb, in_=buv[:].unsqueeze(0))
            bo_sb = consts.tile([1, c.DM], FR, tag="bo")
            wdma(out=bo_sb, in_=bo[:].unsqueeze(0))
            ones2 = consts.tile([128, 2, 128], FP8, tag="ones2")
            wdma(out=ones2, in_=ones_d.rearrange("(two p) m -> p two m", p=128))
            ones1 = consts.tile([1, 128], FR, tag="ones1")
            wdma(out=ones1, in_=ones_bf_d[:, :])

            for st in range(c.STN):
                ssl = bass.ts(st, c.ST)
                xts = []
                for e in range(c.EC):
                    t = p1ax.tile([128, c.ST], FR, tag="xt")
                    adma(out=t, in_=xt[e * 128:(e + 1) * 128, ssl])
                    xts.append(t)
                for ct in range(c.CC):
                    ps = psS.tile([128, c.ST], F32, tag="s")
                    for e in range(c.EC):
                        nc.tensor.matmul(ps, r(wdkv_t[e][:, ct * 128:(ct + 1) * 128]),
                                         r(xts[e]), start=(e == 0), stop=(e == c.EC - 1))
                    nc.vector.tensor_scalar_add(ckvT[:, ct, ssl], ps,
                                                bdkv_sb[:, ct:ct + 1])
                # k_rot: one 128-col stationary covers A rows (0:64, cos
                # part) and companion Ar rows (64:128, sin part) in one psum
                ps = psS.tile([128, c.ST], F32, tag="s")
                for e in range(c.EC):
                    nc.tensor.matmul(ps, r(wrk_t[e]), r(xts[e]),
                                     start=(e == 0), stop=(e == c.EC - 1))
                tmp = p1at.tile([128, c.ST], F32, tag="ktmp")
                nc.vector.scalar_tensor_tensor(tmp[0:64, :], ps[0:64, :],
                                               brk_sb[0:64, :],
                                               coss_sb[0:64, ssl], ALU.add, ALU.mult)
                nc.vector.scalar_tensor_tensor(tmp[64:128, :], ps[64:128, :],
                                               brk_sb[64:128, :],
                                               coss_sb[64:128, ssl], ALU.add, ALU.mult)
                tmp2 = p1at.tile([64, c.ST], F32, tag="ktmp2")
                gdma(out=tmp2, in_=tmp[64:128, :])
                nc.vector.tensor_add(krT[0:64, ssl], tmp[0:64, :], tmp2)
            # duplicate kr rows: rows 64:128 are the stationary rows that
            # multiply qrT2's zero rows - any finite value works, a copy is
            # the cheapest way to guarantee finite.
            gdma(out=krT[64:128, :], in_=krT[0:64, :])

        # zero qrT2's pad rows off the k_rot tail's critical path; the rot
        # matmuls (attention) are the only readers
        nc.gpsimd.memset(qrT2[64:128, :, :], 0.0)

        # pre-issue head 0/1 attention weights: the hw zone aliases nothing,
        # so these flow on the ACT ring during 1b/1c
        wuv_p = hw.tile([128, c.CC, 256], FR, tag="wuv", name="wuv_pre")
        wdma(out=wuv_p,
             in_=wuv.rearrange("(cc p) m -> p cc m", p=128)[:, :, 0:256])
        wuk_p0 = hw.tile([128, c.CC, 128], FR, tag="wuk", name="wuk_pre0")
        wdma(out=wuk_p0,
             in_=wuk.rearrange("(cc p) m -> p cc m", p=128)[:, :, 0:128])
        wuk_p1 = hw.tile([128, c.CC, 128], FR, tag="wuk", name="wuk_pre1")
        wdma(out=wuk_p1,
             in_=wuk.rearrange("(cc p) m -> p cc m", p=128)[:, :, 128:256])

        with tc.tile_pool(name="p1bx", bufs=c.QTN * c.EC + 2) as p1bx, \
             tc.tile_pool(name="p1bw", bufs=2) as p1bw:
            # 1b's activations: fresh zone, so these queue dep-free on the
            # SP ring right behind 1a's xt stream
            xqs = {}
            for qt in range(c.QTN):
                qsl = bass.ts(qt, c.QT)
                for e in range(c.EC):
                    t = p1bx.tile([128, c.QT], FR, tag="xq")
                    adma(out=t, in_=xtq[e * 128:(e + 1) * 128, qsl])
                    xqs[qt, e] = t

            # ==============================================================
            # Phase 1b/1c: c_q^T, then q^T (scaled) and roped q_rot^T
            # ==============================================================
            with tc.tile_pool(name="pcq", bufs=1) as pcq, \
                 tc.tile_pool(name="p1cm", bufs=1) as p1cm, \
                 tc.tile_pool(name="p1cw", bufs=2) as p1cw:
                cqT = pcq.tile([128, c.C1C, c.Q], FR, tag="cqT")

                cosq_sb = p1cm.tile([128, c.Q], F32, tag="cosq")
                sinq_sb = p1cm.tile([128, c.Q], F32, tag="sinq")
                wdma(out=cosq_sb, in_=cosq[:, :])
                wdma(out=sinq_sb, in_=sinq[:, :])

                for ct in range(c.C1C):
                    wdq_ct = p1bw.tile([128, c.EC, 128], FR, tag="wdq")
                    wdma(
                        out=wdq_ct,
                        in_=wdq.rearrange("(e p) m -> p e m", p=128)[:, :, ct * 128:(ct + 1) * 128])
                    for qt in range(c.QTN):
                        qsl = bass.ts(qt, c.QT)
                        ps = psS.tile([128, c.QT], F32, tag="s")
                        for e in range(c.EC):
                            nc.tensor.matmul(ps, r(wdq_ct[:, e, :]), r(xqs[qt, e]),
                                             start=(e == 0), stop=(e == c.EC - 1))
                        nc.vector.tensor_scalar_add(cqT[:, ct, qsl], ps,
                                                    bdq_sb[:, ct:ct + 1])

                with tc.tile_pool(name="p1ct", bufs=4) as p1ct:
                    for h in range(c.H):
                        wuq_h = p1cw.tile([128, c.C1C, 128], FR, tag="wuq")
                        wdma(
                            out=wuq_h,
                            in_=wuq.rearrange("(cc p) m -> p cc m", p=128)[:, :, h * 128:(h + 1) * 128])
                        for qt in range(c.QTN):
                            qsl = bass.ts(qt, c.QT)
                            ps = psS.tile([128, c.QT], F32, tag="s")
                            for ct in range(c.C1C):
                                nc.tensor.matmul(ps, r(wuq_h[:, ct, :]), r(cqT[:, ct, qsl]),
                                                 start=(ct == 0), stop=(ct == c.C1C - 1))
                            nc.vector.tensor_scalar_add(qT[:, h, qsl], ps,
                                                        buq_sb[:, h:h + 1])
                    for hp in range(c.H // 2):
                        wrq_hp = p1cw.tile([128, c.C1C, 128], FR, tag="wrq")
                        wdma(
                            out=wrq_hp,
                            in_=wrq.rearrange("(cc p) m -> p cc m", p=128)[:, :, hp * 128:(hp + 1) * 128])
                        wrqr_hp = p1cw.tile([128, c.C1C, 128], FR, tag="wrqr")
                        wdma(
                            out=wrqr_hp,
                            in_=wrqr.rearrange("(cc p) m -> p cc m", p=128)[:, :, hp * 128:(hp + 1) * 128])
                        for qt in range(c.QTN):
                            qsl = bass.ts(qt, c.QT)
                            psa = psS.tile([128, c.QT], F32, tag="s")
                            for ct in range(c.C1C):
                                nc.tensor.matmul(psa, r(wrq_hp[:, ct, :]), r(cqT[:, ct, qsl]),
                                                 start=(ct == 0), stop=(ct == c.C1C - 1))
                            psar = psS.tile([128, c.QT], F32, tag="s")
                            for ct in range(c.C1C):
                                nc.tensor.matmul(psar, r(wrqr_hp[:, ct, :]), r(cqT[:, ct, qsl]),
                                                 start=(ct == 0), stop=(ct == c.C1C - 1))
                            tmp = p1ct.tile([128, c.QT], F32, tag="qtmp")
                            nc.vector.scalar_tensor_tensor(tmp, psa, brq_sb[:, hp:hp + 1],
                                                           cosq_sb[:, qsl], ALU.add, ALU.mult)
                            qrp = p1ct.tile([128, c.QT], BF, tag="qrp")
                            nc.vector.scalar_tensor_tensor(qrp, psar,
                                                           brqr_sb[:, hp:hp + 1],
                                                           sinq_sb[:, qsl], ALU.add, ALU.mult)
                            nc.vector.tensor_add(qrp, qrp, tmp)
                            # pair-packed rows -> per-head zero-padded layout
                            gdma(out=qrT2[0:64, 2 * hp, qsl], in_=qrp[0:64, :])
                            gdma(out=qrT2[0:64, 2 * hp + 1, qsl], in_=qrp[64:128, :])

        # ==================================================================
        # Phase 2: per-head attention, kT/v produced one head ahead so their
        # PSUM evictions sit before the reciprocals in engine FIFOs (v casts
        # go to the scalar engine for the same reason). Phase 3's first four
        # chains run inside head 15 to cover its tail.
        # ==================================================================
        wo_pre = []
        with tc.tile_pool(name="ow", bufs=c.H + 2) as ow, \
             tc.tile_pool(name="oo", bufs=2) as oo, \
             tc.tile_pool(name="oa", bufs=c.QON) as oa:
            aqs = []

            def p3_chain(wo_t, mt, qo):
                msl = bass.ts(mt, c.MT)
                ps = psS.tile([128, c.MT], F32, tag="s")
                for hc in range(c.H):
                    nc.tensor.matmul(ps, r(aqs[qo][:, hc, :]), r(wo_t[hc]),
                                     start=(hc == 0),
                                     stop=(not has_bo and hc == c.H - 1))
                if has_bo:
                    nc.tensor.matmul(ps, r(ones1), r(bo_sb[:, msl]),
                                     start=False, stop=True)
                osb = oo.tile([128, c.MT], F32, tag="osb")
                nc.vector.tensor_copy(osb, ps)
                adma(out=out[qo * 128:(qo + 1) * 128, msl], in_=osb)

            with tc.tile_pool(name="hk", bufs=2) as hk, \
                 tc.tile_pool(name="hv", bufs=4) as hv, \
                 tc.tile_pool(name="he", bufs=4) as he, \
                 tc.tile_pool(name="hz", bufs=2) as hz, \
                 tc.tile_pool(name="hr", bufs=2) as hr:

                v_tiles, k_tiles = {}, {}

                def produce_v(hp, pre=None):
                    if pre is None:
                        wuv_hp = hw.tile([128, c.CC, 256], FR, tag="wuv")
                        adma(
                            out=wuv_hp,
                            in_=wuv.rearrange("(cc p) m -> p cc m", p=128)[:, :, hp * 256:(hp + 1) * 256])
                    else:
                        wuv_hp = pre
                    v0 = hv.tile([128, c.KC, 128], FR, tag="vh")
                    v1 = hv.tile([128, c.KC, 128], FR, tag="vh")
                    for st in range(c.KC):
                        psf = psA.tile([128, c.NT], F32, tag="ps")
                        ps = psf[:, 0:256]
                        for cc in range(c.CC):
                            nc.tensor.matmul(ps, r(ckvT[:, cc, st * 128:(st + 1) * 128]),
                                             r(wuv_hp[:, cc, :]),
                                             start=(cc == 0),
                                             stop=(not has_buv and cc == c.CC - 1))
                        if has_buv:
                            nc.tensor.matmul(ps, r(ones1),
                                             r(buv_sb[:, hp * 256:(hp + 1) * 256]),
                                             start=False, stop=True)
                        nc.scalar.copy(v0[:, st, :], ps[:, 0:128])
                        nc.scalar.copy(v1[:, st, :], ps[:, 128:256])
                    v_tiles[2 * hp], v_tiles[2 * hp + 1] = v0, v1

                def produce_k(h, pre=None):
                    if pre is None:
                        wuk_h = hw.tile([128, c.CC, 128], FR, tag="wuk")
                        adma(
                            out=wuk_h,
                            in_=wuk.rearrange("(cc p) m -> p cc m", p=128)[:, :, h * 128:(h + 1) * 128])
                    else:
                        wuk_h = pre
                    kT = hk.tile([128, c.S], FR, tag="kT")
                    for nt in range(c.NTN):
                        nsl = bass.ts(nt, c.NT)
                        ps = psA.tile([128, c.NT], F32, tag="ps")
                        for cc in range(c.CC):
                            nc.tensor.matmul(ps, r(wuk_h[:, cc, :]), r(ckvT[:, cc, nsl]),
                                             start=(cc == 0), stop=(cc == c.CC - 1))
                        nc.scalar.add(kT[:, nsl], ps, buk_sb[:, h:h + 1])
                    k_tiles[h] = kT

                produce_v(0, pre=wuv_p)
                produce_k(0, pre=wuk_p0)
                for h in range(c.H):
                    if h % 2 == 0 and h + 2 < c.H:
                        produce_v(h // 2 + 1)
                    if h + 1 < c.H:
                        produce_k(h + 1, pre=wuk_p1 if h == 0 else None)
                    # one wo prefetch per head for phase 3's mt=0
                    t = ow.tile([128, c.MT], FR, tag="wo")
                    adma(out=t, in_=wo[h * 128:(h + 1) * 128, 0:c.MT])
                    wo_pre.append(t)

                    kT = k_tiles.pop(h)
                    vh = v_tiles.pop(h)
                    for qt in range(c.QTN):
                        qsl = bass.ts(qt, c.QT)
                        gps = psG.tile([128, c.QT], F32, tag="g")
                        zps = psZ.tile([128, c.QT], F32, tag="z")
                        ets, ess = [], []
                        es2 = None
                        for kc in range(c.KC):
                            ksl = bass.ts(kc, 128)
                            sps = psS.tile([128, c.QT], F32, tag="s")
                            nc.tensor.matmul(sps, r(kT[:, ksl]), r(qT[:, h, qsl]),
                                             start=True, stop=False)
                            nc.tensor.matmul(sps, r(krT[:, ksl]),
                                             r(qrT2[:, h, qsl]),
                                             start=False, stop=True)
                            et = he.tile([128, c.QT], FR, tag="e")
                            nc.scalar.activation(et, sps, AF.Exp)
                            nc.tensor.matmul(gps, r(vh[:, kc, :]), r(et),
                                             start=(kc == 0), stop=(kc == c.KC - 1))
                            ets.append(et)
                            if kc % 2 == 1:
                                # denominator at 1/4 the PE cost: the same 8
                                # Pool pair-sums as before, but written in fp8
                                # into 2-slot tiles so one fp8 DoubleRow
                                # ones-matmul covers FOUR key chunks. Matmuls
                                # are emitted one slot-pair late so the PE
                                # FIFO never waits on a fresh pair-sum.
                                if kc % 4 == 1:
                                    es2 = hz.tile([128, 2, c.QT], FP8,
                                                  tag="es2")
                                nc.gpsimd.tensor_add(es2[:, (kc % 4) // 2, :],
                                                     ets[kc - 1], ets[kc])
                                if kc % 4 == 3:
                                    ess.append(es2)
                                    if len(ess) >= 2:
                                        i = len(ess) - 2
                                        nc.tensor.matmul(zps, ones2, ess[i],
                                                         start=(i == 0),
                                                         stop=False,
                                                         perf_mode=DRW)
                        i = len(ess) - 1
                        nc.tensor.matmul(zps, ones2, ess[i], start=(i == 0),
                                         stop=True, perf_mode=DRW)
                        recip = hr.tile([128, c.QT], F32, tag="recip")
                        nc.vector.reciprocal(recip, zps)
                        asb = hr.tile([128, c.QT], BF, tag="attsb")
                        nc.vector.tensor_mul(asb, gps, recip)
                        adma(out=attd[h * 128:(h + 1) * 128, qsl], in_=asb)
                        if h == c.H - 1 and qt == 0:
                            # every head's qt0 slab is in attd: prefetch
                            # phase 3's first stationaries under qt1's work
                            for qo in range(c.QON // 2):
                                aq = oa.tile([128, c.H, 128], BF, tag="attq")
                                adma(
                                    out=aq,
                                    in_=attd.rearrange("(hc p) q -> p hc q", p=128)[:, :, qo * 128:(qo + 1) * 128])
                                aqs.append(aq)

            # ==============================================================
            # Phase 3: output projection  out[q, m] = attT.T @ wo + bo
            # ==============================================================
            for qo in range(c.QON // 2, c.QON):
                aq = oa.tile([128, c.H, 128], BF, tag="attq")
                adma(
                    out=aq,
                    in_=attd.rearrange("(hc p) q -> p hc q", p=128)[:, :, qo * 128:(qo + 1) * 128])
                aqs.append(aq)
            for mt in range(c.MTN):
                if mt == 0:
                    wo_t = wo_pre
                    qos = range(c.QON)
                else:
                    wo_t = []
                    for hc in range(c.H):
                        t = ow.tile([128, c.MT], FR, tag="wo")
                        adma(out=t, in_=wo[hc * 128:(hc + 1) * 128, bass.ts(mt, c.MT)])
                        wo_t.append(t)
                    qos = range(c.QON)
                for qo in qos:
                    p3_chain(wo_t, mt, qo)

    return nc


# ----------------------------------------------------------------------------
# Host side: input prep, sharding, gather
# ----------------------------------------------------------------------------

def _rope_tables(seq_len, dim, theta=10000.0):
    inv_freq = 1.0 / (theta ** (np.arange(0, dim, 2, dtype=np.float32) / dim))
    t = np.arange(seq_len, dtype=np.float32)
    ang = t[:, None] * inv_freq[None, :]  # [S, dim/2]
    return np.cos(ang).astype(np.float32), np.sin(ang).astype(np.float32)


def _rot_companion_cols(w):
    """wr[..., 2i] = -w[..., 2i+1]; wr[..., 2i+1] = w[..., 2i]."""
    wr = np.empty_like(w)
    wr[..., 0::2] = -w[..., 1::2]
    wr[..., 1::2] = w[..., 0::2]
    return wr


def host_inputs(cfg, sequence, W_dkv, b_dkv, W_dq, b_dq, W_uq, b_uq, W_uk, b_uk,
                W_uv, b_uv, W_rq, b_rq, W_rk, b_rk, W_o, b_o):
    """Build the per-core input maps for the SPMD program."""
    c = cfg
    f = lambda a: np.ascontiguousarray(np.asarray(a, dtype=np.float32))  # noqa: E731
    sequence = f(sequence)
    B = sequence.shape[0]
    scaler = np.float32(1.0 / np.sqrt(c.DH + c.DR))

    cos, sin = _rope_tables(c.S, c.DR)  # [S, 32]
    # rows 2i and 2i+1 both carry table column i
    cosk = np.repeat(cos.T, 2, axis=0)  # [64, S]
    sink = np.repeat(sin.T, 2, axis=0)

    shared = dict(
        wdq=f(W_dq), bdq=f(b_dq),
        wdkv=f(W_dkv), bdkv=f(b_dkv),
        wuq=f(W_uq) * scaler, buq=f(b_uq) * scaler,
        wrq=f(W_rq) * scaler, brq=f(b_rq) * scaler,
        wrqr=_rot_companion_cols(f(W_rq) * scaler),
        brqr=_rot_companion_cols(f(b_rq) * scaler),
        wrk=np.concatenate([f(W_rk), _rot_companion_cols(f(W_rk))], axis=1),
        brk=np.concatenate([f(b_rk), _rot_companion_cols(f(b_rk))], axis=0),
        wuk=f(W_uk), buk=f(b_uk),
        wuv=f(W_uv), buv=f(b_uv),
        wo=f(W_o), bo=f(b_o),
        cossink=np.concatenate([f(cosk), f(sink)], axis=0),
        ones_in=np.ones((256, 128), np.float32),
        ones_bf=np.ones((1, 128), np.float32),
    )
    shared = {k: np.ascontiguousarray(v) for k, v in shared.items()}
    mm_keys = {"wdq", "wdkv", "wuq", "wrq", "wrqr", "wrk", "wuk", "wuv", "wo",
               "buv", "bo", "ones_bf"}
    if getattr(c, "bf16", False):
        import ml_dtypes
        for k in mm_keys:
            shared[k] = shared[k].astype(ml_dtypes.bfloat16)
    import ml_dtypes
    shared["ones_in"] = shared["ones_in"].astype(ml_dtypes.float8_e4m3fn)

    n_cores = 2 * B
    in_maps = []
    for core in range(n_cores):
        b, half = core // 2, core % 2
        xtc = np.ascontiguousarray(sequence[b].T)         # [E, S]
        q0 = half * c.Q
        xtqc = np.ascontiguousarray(xtc[:, q0:q0 + c.Q])  # [E, Q]
        cq = np.tile(np.repeat(cos[q0:q0 + c.Q].T, 2, axis=0), (2, 1))  # [128, Q]
        sq = np.tile(np.repeat(sin[q0:q0 + c.Q].T, 2, axis=0), (2, 1))
        m = dict(shared)
        if getattr(c, "bf16", False):
            import ml_dtypes
            xtc = xtc.astype(ml_dtypes.bfloat16)
            xtqc = xtqc.astype(ml_dtypes.bfloat16)
        m.update(xt=xtc, xtq=xtqc,
                 cosq=np.ascontiguousarray(cq), sinq=np.ascontiguousarray(sq))
        in_maps.append(m)
    return in_maps


_PROG_CACHE = {}


def kernel(**inputs) -> np.ndarray:
    from concourse.bass_utils import run_bass_kernel_spmd

    _install_wait_split_hook()

    cfg = FULL
    has_buv = bool(np.any(np.asarray(inputs["b_uv"])))
    has_bo = bool(np.any(np.asarray(inputs["b_o"])))
    key = ("full", has_buv, has_bo)
    if key not in _PROG_CACHE:
        _PROG_CACHE[key] = build_program(cfg, has_buv=has_buv, has_bo=has_bo)
    nc = _PROG_CACHE[key]

    in_maps = host_inputs(cfg, **inputs)
    n = len(in_maps)
    res = run_bass_kernel_spmd(nc, in_maps, list(range(n)))

    B = n // 2
    S = 2 * cfg.Q
    full = np.empty((B, S, cfg.DM), dtype=np.float32)
    for core in range(n):
        b, half = core // 2, core % 2
        full[b, half * cfg.Q:(half + 1) * cfg.Q, :] = res.results[core]["out"]
    return full

